# revision 1
# baseline (speedup 1.0000x reference)
"""Trainium2 Bass kernel for nn_AudioTransformer (neighborhood-attention transformer).

Strategy: sequence-parallel over 8 NeuronCores (64 tokens/core). Weights are
replicated per core in bf16 and streamed layer-by-layer (double-buffered).
Activations live feature-major (features on SBUF partitions, tokens on the
free dim) so the whole layer stack runs without a single on-chip transpose.
Neighborhood attention is computed dense over all 512 keys with a
host-precomputed bias table (rel-pos bias inside the clamped window, -60
outside), keys-on-partitions so the softmax key-reduction is a ones-matmul,
and softmax skips max-subtraction (logits provably in [-2, 2]).
Per layer, one 8-core AllGather shares each core's bf16 K/V slab.
"""

import numpy as np
import ml_dtypes

import concourse.bass as bass
import concourse.mybir as mybir
import concourse.tile as tile
from concourse.tile import add_dep_helper
from concourse import bacc
from concourse.bass_utils import run_bass_kernel_spmd


def _install_act_table_filter():
    """Make the act-table chooser resolve Ln/Exp/Identity/Copy only via the
    natural_log_exp_and_others set so each layer needs just 2 LUT swaps
    (to gelu_and_others and back) instead of 5. Positional set ids are
    preserved; sets are only shrunk, so every emitted load is still valid."""
    import concourse.bacc as _bacc_mod
    if getattr(_bacc_mod, "_ant_act_filter", False):
        return
    _orig = _bacc_mod.get_activation_tables
    A = mybir.ActivationFunctionType
    movable = {A.Ln, A.Exp, A.Identity, A.Copy}

    def _filtered(arch):
        t = _orig(arch)
        out = {}
        for name, funcs in t.items():
            if name == "natural_log_exp_and_others":
                out[name] = set(funcs)
            else:
                out[name] = set(funcs) - movable
        return out

    _bacc_mod.get_activation_tables = _filtered
    _bacc_mod._ant_act_filter = True

BF = ml_dtypes.bfloat16
F32 = mybir.dt.float32
BF16 = mybir.dt.bfloat16

NC = 8          # cores
L = 512         # total tokens
LC = L // NC    # tokens per core = 64
D = 512         # model dim
DT = D // 128   # 4 feature tiles
H = 8           # heads
DH = 64         # head dim
DFF = 2048      # ff dim
FT = DFF // 128  # 16 ff tiles
PATCH = 32
LAYERS = 8
K = 127         # neighborhood size
WKEYS = 256     # per-core key window (2 key-tiles, covers all clamped windows)
WKT = WKEYS // 128
NEG = -60.0     # out-of-window logit bias (exp(-60+2) == 0 in fp32/bf16)

# wblob column offsets (per 128-row partition, bf16)
OFF_QKV = 0            # 4 fi-tiles x 1536
OFF_PROJ = 6144        # 4 fi-tiles x 512
OFF_FF1 = 8192         # 4 fi-tiles x 2048
OFF_FF2 = 16384        # 16 fi-tiles x 512
WCOLS = 24576

# pblob columns (f32)
PB_QKVB = 0    # 12
PB_PROJB = 12  # 4
PB_FF1B = 16   # 16
PB_FF2B = 32   # 4
PB_LN1G = 36
PB_LN1B = 40
PB_LN2G = 44
PB_LN2B = 48
PCOLS = 52

_BUILD_CACHE = {}


def _build(repeat=1):
    """Build + finalize the SPMD Bass graph (same graph on all 8 cores)."""
    _install_act_table_filter()
    nc = bacc.Bacc(None, target_bir_lowering=False)

    # ---- DRAM parameters (per-core inputs) ----
    xT = nc.dram_tensor("xT", [PATCH, LC], BF16, kind="ExternalInput")
    w_in_T = nc.dram_tensor("w_in_T", [PATCH, D], BF16, kind="ExternalInput")
    inb = nc.dram_tensor("inb", [128, DT], F32, kind="ExternalInput")
    wblob = nc.dram_tensor("wblob", [LAYERS, 128, WCOLS], BF16, kind="ExternalInput")
    pblob = nc.dram_tensor("pblob", [LAYERS, 128, PCOLS], F32, kind="ExternalInput")
    vbias = nc.dram_tensor("vbias", [LAYERS, D], BF16, kind="ExternalInput")
    fbrow = nc.dram_tensor("fbrow", [LAYERS, DFF], BF16, kind="ExternalInput")
    bmask = nc.dram_tensor("bmask", [LAYERS, H, WKEYS, LC], BF16, kind="ExternalInput")
    w_out = nc.dram_tensor("w_out", [128, 128], BF16, kind="ExternalInput")
    outb = nc.dram_tensor("outb", [PATCH, 1], F32, kind="ExternalInput")
    yT = nc.dram_tensor("yT", [PATCH, LC], F32, kind="ExternalOutput")

    with tile.TileContext(nc) as tc:
        with (
            tc.tile_pool(name="singles", bufs=1) as singles,
            tc.tile_pool(name="wpool", bufs=2) as wpool,
            tc.tile_pool(name="ppool", bufs=2) as ppool,
            tc.tile_pool(name="bmpool", bufs=2) as bmpool,
            tc.tile_pool(name="actpool", bufs=2) as actpool,
            tc.tile_pool(name="gatherpool", bufs=2) as gatherpool,
            tc.tile_pool(name="tmppool", bufs=3) as tmppool,
            tc.tile_pool(name="statpool", bufs=4) as statpool,
            tc.tile_pool(name="agdram", bufs=2, space="DRAM") as agdram,
            # PSUM: 8 banks total, every tile slot rounds to one bank.
            # pp:mm_out(3) + pp_ln:sums(1) + ppv(1) + ppatt:ps_l(2) + ppbc:bcast(1) = 8
            tc.tile_pool(name="pp", bufs=3, space="PSUM") as pp,
            tc.tile_pool(name="pp_ln", bufs=1, space="PSUM") as pp_ln,
            tc.tile_pool(name="ppv", bufs=1, space="PSUM") as ppv,
            tc.tile_pool(name="ppatt", bufs=2, space="PSUM") as ppatt,
            tc.tile_pool(name="ppbc", bufs=1, space="PSUM") as ppbc,
        ):
            # persistent tiles
            hT = singles.tile([128, DT, LC], F32)          # residual stream h.T
            ones_f = singles.tile([128, 1], F32)
            ones_b = singles.tile([128, 1], BF16)
            ones_row = singles.tile([1, 128], BF16)
            ones_bcf = singles.tile([1, 128], F32)
            xin = singles.tile([PATCH, LC], BF16)
            win = singles.tile([PATCH, D], BF16)
            inb_s = singles.tile([128, DT], F32)
            wout_s = singles.tile([128, 128], BF16)
            outb_s = singles.tile([PATCH, 1], F32)

            nc.vector.memset(ones_f[:], 1.0)
            nc.vector.memset(ones_b[:], 1.0)
            nc.vector.memset(ones_row[:], 1.0)
            nc.vector.memset(ones_bcf[:], 1.0)
            nc.sync.dma_start(xin[:], xT[:])
            nc.sync.dma_start(win[:], w_in_T[:])
            nc.sync.dma_start(inb_s[:], inb[:])
            nc.sync.dma_start(wout_s[:], w_out[:])
            nc.sync.dma_start(outb_s[:], outb[:])

            def layernorm(src, gcol, bcol, pb, dst):
                """src [128,DT,LC] f32 -> dst [128,DT,LC] bf16 (normalized*g+b)."""
                sq = tmppool.tile([128, DT, LC], F32, tag="ln_sq")
                nc.vector.tensor_mul(sq[:], src[:], src[:])
                ps_s = pp_ln.tile([1, 2 * LC], F32, tag="sums", name="ps_s")
                for f in range(DT):
                    nc.tensor.matmul(ps_s[0:1, 0:LC], ones_f[:], src[:, f, :],
                                     start=(f == 0), stop=(f == DT - 1))
                for f in range(DT):
                    nc.tensor.matmul(ps_s[0:1, LC:2 * LC], ones_f[:], sq[:, f, :],
                                     start=(f == 0), stop=(f == DT - 1))
                st = statpool.tile([1, 2 * LC], F32, tag="ln_st")
                nc.vector.tensor_scalar_mul(st[0:1, 0:LC], ps_s[0:1, 0:LC], 1.0 / D)
                # (m2 - eps) elementwise, then var+eps = sumsq/D - (m2 - eps), fused
                m2 = statpool.tile([1, LC], F32, tag="ln_m2")
                nc.vector.tensor_mul(m2[:], st[0:1, 0:LC], st[0:1, 0:LC])
                nc.vector.tensor_scalar_add(m2[:], m2[:], -1e-5)
                var = statpool.tile([1, LC], F32, tag="ln_var")
                nc.vector.scalar_tensor_tensor(
                    var[:], ps_s[0:1, LC:2 * LC], 1.0 / D, m2[:],
                    op0=mybir.AluOpType.mult, op1=mybir.AluOpType.subtract)
                # rstd = exp(-0.5*ln(var)) -- keeps ACT in the Ln/Exp func set
                sd = statpool.tile([1, LC], F32, tag="ln_sd")
                nc.scalar.activation(sd[:], var[:], mybir.ActivationFunctionType.Ln)
                nc.scalar.activation(st[0:1, LC:2 * LC], sd[:],
                                     mybir.ActivationFunctionType.Exp, scale=-0.5)
                # broadcast (mean, rstd) across all 128 partitions via K=1 matmul
                bc = ppbc.tile([128, 2 * LC], F32, tag="bcast", name="bc")
                nc.tensor.matmul(bc[:], ones_bcf[:], st[:], start=True, stop=True)
                t0 = tmppool.tile([128, DT, LC], F32, tag="ln_t0")
                mean_w = bc[:, 0:LC].unsqueeze(1).to_broadcast([128, DT, LC])
                rstd_w = bc[:, LC:2 * LC].unsqueeze(1).to_broadcast([128, DT, LC])
                nc.vector.tensor_sub(t0[:], src[:], mean_w)
                # gamma/beta are folded into the consumer matmul weights on the
                # host, so plain normalize writes the bf16 output directly
                nc.vector.tensor_mul(dst[:], t0[:], rstd_w)

            # ---- input projection: h0.T = in_w @ x_slice.T + in_b ----
            for t in range(DT):
                ps = pp.tile([128, LC], F32, tag="mm_out")
                nc.tensor.matmul(ps[:], win[:, t * 128:(t + 1) * 128], xin[:],
                                 start=True, stop=True)
                nc.scalar.activation(hT[:, t, :], ps[:],
                                     mybir.ActivationFunctionType.Identity,
                                     bias=inb_s[:, t:t + 1], scale=1.0)

            # per-core 256-key window start rank r0 = clip(rank-2, 0, 4),
            # as branch-free register arithmetic for dynamic DMA offsets
            rank = nc.sync.partition_id()
            ind36 = (rank >= 3) & (rank <= 6)
            ind7 = rank >= 7
            r0v = (rank - 2) * ind36 + 4 * ind7

            def load_layer(l, after=None):
                w_qkv = wpool.tile([128, 6144], BF16, tag="w_qkv", name="w_qkv")
                w_proj = wpool.tile([128, 2048], BF16, tag="w_proj", name="w_proj")
                w_ff1 = wpool.tile([128, 8192], BF16, tag="w_ff1", name="w_ff1",
                                   bufs=3)
                w_ff2 = wpool.tile([128, 8192], BF16, tag="w_ff2", name="w_ff2",
                                   bufs=3)
                pb = ppool.tile([128, PCOLS], F32, tag="pb", name="pb")
                fb = ppool.tile([1, DFF], BF16, tag="fb", name="fb")
                bm = bmpool.tile([128, H, WKT, LC], BF16, tag="bm", name="bm")
                vb = ppool.tile([1, D], BF16, tag="vb", name="vb")
                ds_ = [
                    nc.sync.dma_start(w_qkv[:], wblob[l, :, OFF_QKV:OFF_PROJ]),
                    nc.sync.dma_start(w_proj[:], wblob[l, :, OFF_PROJ:OFF_FF1]),
                    nc.sync.dma_start(w_ff1[:], wblob[l, :, OFF_FF1:OFF_FF2]),
                    nc.sync.dma_start(w_ff2[:], wblob[l, :, OFF_FF2:WCOLS]),
                    nc.sync.dma_start(pb[:], pblob[l]),
                    nc.sync.dma_start(
                        bm[:], bmask[l].rearrange("h (kt p) q -> p h kt q", p=128)),
                    nc.sync.dma_start(vb[:], vbias[l].unsqueeze(0)),
                    nc.sync.dma_start(fb[:], fbrow[l].unsqueeze(0)),
                ]
                if after is not None:
                    # keep next-layer transfers off the DMA device until this
                    # layer's collective input write has gone through: they
                    # stream during the collective instead of delaying it
                    for d in ds_:
                        add_dep_helper(d.ins, after.ins, sync=True,
                                       reason="layer prefetch after ag write")
                return w_qkv, w_proj, w_ff1, w_ff2, pb, bm, vb, fb

            for rep in range(repeat):
                cur = load_layer(0)
                for l in range(LAYERS):
                    w_qkv, w_proj, w_ff1, w_ff2, pb, bm, vb, fb = cur

                    # ---- LN1 ----
                    xb = actpool.tile([128, DT, LC], BF16, tag="xb")
                    layernorm(hT, PB_LN1G, PB_LN1B, pb, xb)

                    # ---- QKV ----
                    # ---- AllGather x~ (bf16, half the K+V payload); each core
                    # recomputes K,V for its 256-key window locally (bit-identical
                    # math, ~3.7us of spare PE) ----
                    ag_in = agdram.tile([D * LC], BF16, tag="ag_in")
                    ag_out = agdram.tile([NC, D * LC], BF16, tag="ag_out",
                                         addr_space="Shared")
                    ag_w = nc.sync.dma_start(
                        ag_in[:].rearrange("(f p t) -> p f t", p=128, t=LC),
                        xb[:])
                    nc.gpsimd.collective_compute(
                        "AllGather", mybir.AluOpType.bypass,
                        ins=[ag_in[:].opt()], outs=[ag_out[:].opt()],
                        replica_groups=[list(range(NC))])
                    # prefetch next layer's weights NOW: their transfers overlap
                    # this layer's collective instead of queueing behind the
                    # post-collective reads (SP stream head-of-line).
                    if l + 1 < LAYERS:
                        cur = load_layer(l + 1, after=ag_w)
                    # gathered x~ window, feature-major, one tile per fi-tile
                    xw = []
                    for g in range(DT):
                        xw_g = gatherpool.tile([128, WKEYS], BF16, tag=f"xw{g}",
                                               name="xw_g")
                        nc.sync.dma_start(
                            xw_g[:].rearrange("p (r t) -> p r t", t=LC),
                            ag_out[bass.ds(r0v, 4),
                                   g * 128 * LC:(g + 1) * 128 * LC]
                            .rearrange("r (p t) -> p r t", t=LC))
                        xw.append(xw_g)

                    qT = []
                    for t in range(DT):
                        ps = pp.tile([128, LC], F32, tag="mm_out")
                        for f in range(DT):
                            nc.tensor.matmul(
                                ps[:],
                                w_qkv[:, f * 1536 + t * 128:f * 1536 + (t + 1) * 128],
                                xb[:, f, :], start=(f == 0), stop=(f == DT - 1))
                        qT_t = actpool.tile([128, LC], BF16, tag=f"qT{t}", name="qT_t")
                        nc.vector.tensor_scalar_add(
                            qT_t[:], ps[:],
                            pb[:, PB_QKVB + t:PB_QKVB + t + 1])
                        qT.append(qT_t)

                    # K.T window tiles [128=(hh,dh), 256 keys], one per head-pair
                    KTg = []
                    for g in range(DT):
                        ps = ppatt.tile([128, WKEYS], F32, tag="ps_l", name="ps_kw")
                        for f in range(DT):
                            nc.tensor.matmul(
                                ps[:],
                                w_qkv[:, f * 1536 + 512 + g * 128:
                                      f * 1536 + 512 + (g + 1) * 128],
                                xw[f][:], start=(f == 0), stop=(f == DT - 1))
                        KTg_g = gatherpool.tile([128, WKEYS], BF16, tag=f"KTg{g}",
                                                name="KTg_g")
                        nc.vector.tensor_scalar_add(
                            KTg_g[:], ps[:],
                            pb[:, PB_QKVB + DT + g:PB_QKVB + DT + g + 1])
                        KTg.append(KTg_g)
                    # V window token-major tiles [128=tok, D], one per key-tile
                    Vt = []
                    for kt in range(WKT):
                        ps_v = ppv.tile([128, D], F32, tag="ps_v")
                        for f in range(DT):
                            nc.tensor.matmul(
                                ps_v[:], xw[f][:, kt * 128:(kt + 1) * 128],
                                w_qkv[:, f * 1536 + 1024:f * 1536 + 1536],
                                start=(f == 0), stop=False)
                        nc.tensor.matmul(ps_v[:], ones_row[:], vb[:],
                                         start=False, stop=True)
                        Vt_kt = gatherpool.tile([128, D], BF16, tag=f"Vt{kt}",
                                                name="Vt_kt")
                        nc.scalar.copy(Vt_kt[:], ps_v[:])
                        Vt.append(Vt_kt)

                    # ---- attention (per-head tiles so sums/AV/proj pipeline) ----
                    probs = []
                    for h in range(H):
                        hh, g = h % 2, h // 2
                        ps_l = ppatt.tile([128, WKT, LC], F32, tag="ps_l")
                        for kt in range(WKT):
                            nc.tensor.matmul(
                                ps_l[:, kt, :],
                                KTg[g][hh * DH:(hh + 1) * DH,
                                       kt * 128:(kt + 1) * 128],
                                qT[g][hh * DH:(hh + 1) * DH, :],
                                start=True, stop=True)
                        tmp_l = tmppool.tile([128, WKT, LC], F32, tag="att_tmp")
                        nc.vector.tensor_add(tmp_l[:], ps_l[:], bm[:, h, :, :])
                        probs_h = actpool.tile([128, WKT, LC], BF16, tag=f"probs{h}",
                                               name="probs_h")
                        nc.scalar.activation(probs_h[:], tmp_l[:],
                                             mybir.ActivationFunctionType.Exp)
                        probs.append(probs_h)
                    # denominators
                    ps_sum = pp_ln.tile([1, H * LC], F32, tag="sums", name="ps_sum")
                    for h in range(H):
                        for kt in range(WKT):
                            nc.tensor.matmul(ps_sum[0:1, h * LC:(h + 1) * LC],
                                             ones_b[:], probs[h][:, kt, :],
                                             start=(kt == 0), stop=(kt == WKT - 1))
                    rsum = statpool.tile([1, H * LC], F32, tag="rsum")
                    nc.vector.reciprocal(rsum[:], ps_sum[:])
                    rs_ps = ppbc.tile([DH, H * LC], F32, tag="bcast", name="rs_ps")
                    nc.tensor.matmul(rs_ps[:], ones_bcf[0:1, 0:DH], rsum[:],
                                     start=True, stop=True)
                    rs_bc = tmppool.tile([DH, H, LC], F32, tag="rs_bc")
                    nc.vector.tensor_copy(rs_bc[:], rs_ps[:].rearrange("p (h q) -> p h q", q=LC))
                    # AV, one output tile per head-pair
                    oT = [actpool.tile([128, LC], BF16, tag=f"oT{g}", name="oT_g")
                          for g in range(DT)]
                    for h in range(H):
                        hh, g = h % 2, h // 2
                        ps_o = pp.tile([DH, LC], F32, tag="mm_out", name="ps_o")
                        for kt in range(WKT):
                            nc.tensor.matmul(ps_o[:],
                                             Vt[kt][:, h * DH:(h + 1) * DH],
                                             probs[h][:, kt, :],
                                             start=(kt == 0), stop=(kt == WKT - 1))
                        nc.vector.tensor_mul(
                            oT[g][hh * DH:(hh + 1) * DH, :], ps_o[:],
                            rs_bc[:, h, :])

                    # ---- proj + residual ----
                    for t in range(DT):
                        ps = pp.tile([128, LC], F32, tag="mm_out")
                        for f in range(DT):
                            nc.tensor.matmul(
                                ps[:],
                                w_proj[:, f * 512 + t * 128:f * 512 + (t + 1) * 128],
                                oT[f][:], start=(f == 0), stop=(f == DT - 1))
                        nc.vector.scalar_tensor_tensor(
                            hT[:, t, :], ps[:], pb[:, PB_PROJB + t:PB_PROJB + t + 1],
                            hT[:, t, :], op0=mybir.AluOpType.add,
                            op1=mybir.AluOpType.add)

                    # ---- LN2 ----
                    zb = actpool.tile([128, DT, LC], BF16, tag="zb")
                    layernorm(hT, PB_LN2G, PB_LN2B, pb, zb)

                    # ---- FF1 + gelu (z1 split in two tiles so FF2 can start
                    # accumulating after the first half) ----
                    FH = FT // 2
                    z1a = actpool.tile([128, FH, LC], BF16, tag="z1a")
                    z1b = actpool.tile([128, FH, LC], BF16, tag="z1b")
                    for tq in range(FT // 4):
                        ps = pp.tile([128, 4, LC], F32, tag="mm_out", name="ps_ff1")
                        for tt in range(4):
                            t = tq * 4 + tt
                            for f in range(DT):
                                nc.tensor.matmul(
                                    ps[:, tt, :],
                                    w_ff1[:, f * 2048 + t * 128:
                                          f * 2048 + (t + 1) * 128],
                                    zb[:, f, :], start=(f == 0), stop=False)
                            # per-feature bias injected via K=1 matmul so the
                            # gelu below can cover 4 fo-tiles in one ACT op
                            nc.tensor.matmul(
                                ps[:, tt, :],
                                fb[0:1, t * 128:(t + 1) * 128],
                                ones_row[0:1, 0:LC], start=False, stop=True)
                        z1d = z1a if tq < 2 else z1b
                        nc.scalar.activation(
                            z1d[:, (tq % 2) * 4:(tq % 2) * 4 + 4, :], ps[:],
                            mybir.ActivationFunctionType.Gelu)

                    # ---- FF2 + residual ----
                    for t in range(DT):
                        ps = pp.tile([128, LC], F32, tag="mm_out")
                        for g in range(FT):
                            z1d = z1a if g < FH else z1b
                            nc.tensor.matmul(
                                ps[:],
                                w_ff2[:, g * 512 + t * 128:g * 512 + (t + 1) * 128],
                                z1d[:, g % FH, :], start=(g == 0), stop=(g == FT - 1))
                        nc.vector.scalar_tensor_tensor(
                            hT[:, t, :], ps[:], pb[:, PB_FF2B + t:PB_FF2B + t + 1],
                            hT[:, t, :], op0=mybir.AluOpType.add,
                            op1=mybir.AluOpType.add)

            # ---- output projection: y.T = tanh(out_w @ h.T + out_b) ----
            hb = actpool.tile([128, DT, LC], BF16, tag="hb")
            nc.vector.tensor_copy(hb[:], hT[:])
            ps_y = pp.tile([PATCH, LC], F32, tag="mm_out", name="ps_y")
            for f in range(DT):
                nc.tensor.matmul(ps_y[:], wout_s[:, f * PATCH:(f + 1) * PATCH],
                                 hb[:, f, :], start=(f == 0), stop=(f == DT - 1))
            y_sb = actpool.tile([PATCH, LC], F32, tag="y_sb")
            nc.scalar.activation(y_sb[:], ps_y[:],
                                 mybir.ActivationFunctionType.Tanh,
                                 bias=outb_s[:, 0:1], scale=1.0)
            nc.sync.dma_start(yT[:], y_sb[:])

    nc.finalize()
    return nc


def _prep_inputs(inputs):
    """Host-side: pack full fp32 inputs into per-core in_maps."""
    I = {k: np.asarray(v, np.float32) for k, v in inputs.items()}

    scale = np.float32(DH ** -0.5)
    qkv_w = I["qkv_w"].copy()          # [LAYERS, 3D, D]
    qkv_b = I["qkv_b"].copy()          # [LAYERS, 3D]
    ff1_w = I["ff1_w"].copy()          # [LAYERS, DFF, D]
    ff1_b = I["ff1_b"].copy()          # [LAYERS, DFF]
    # fold LN affines into the consuming matmuls (exact algebra, fp32):
    # (xn*g + b) @ W.T = xn @ (W*diag(g)).T + W@b
    for l in range(LAYERS):
        qkv_b[l] += qkv_w[l] @ I["ln1_b"][l]
        qkv_w[l] *= I["ln1_g"][l][None, :]
        ff1_b[l] += ff1_w[l] @ I["ln2_b"][l]
        ff1_w[l] *= I["ln2_g"][l][None, :]
    qkv_w[:, :D] *= scale
    qkv_b[:, :D] *= scale

    def part_major(m):
        # [X*128, Y] -> [128, X*Y] with column blocks per 128-row tile
        X = m.shape[0] // 128
        return np.ascontiguousarray(
            m.reshape(X, 128, m.shape[1]).transpose(1, 0, 2).reshape(128, -1))

    wblob = np.empty((LAYERS, 128, WCOLS), BF)
    pblob = np.empty((LAYERS, 128, PCOLS), np.float32)
    for l in range(LAYERS):
        qkvT = np.ascontiguousarray(qkv_w[l].T)          # [D, 3D]
        projT = np.ascontiguousarray(I["proj_w"][l].T)   # [D, D]
        ff1T = np.ascontiguousarray(ff1_w[l].T)          # [D, DFF]
        ff2T = np.ascontiguousarray(I["ff2_w"][l].T)     # [DFF, D]
        wblob[l, :, OFF_QKV:OFF_PROJ] = part_major(qkvT).astype(BF)
        wblob[l, :, OFF_PROJ:OFF_FF1] = part_major(projT).astype(BF)
        wblob[l, :, OFF_FF1:OFF_FF2] = part_major(ff1T).astype(BF)
        wblob[l, :, OFF_FF2:WCOLS] = part_major(ff2T).astype(BF)
        pblob[l, :, PB_QKVB:PB_QKVB + 12] = qkv_b[l].reshape(12, 128).T
        pblob[l, :, PB_PROJB:PB_PROJB + 4] = I["proj_b"][l].reshape(4, 128).T
        pblob[l, :, PB_FF1B:PB_FF1B + 16] = ff1_b[l].reshape(16, 128).T  # unused on-device now
        pblob[l, :, PB_FF2B:PB_FF2B + 4] = I["ff2_b"][l].reshape(4, 128).T
        pblob[l, :, PB_LN1G:PB_LN1G + 4] = I["ln1_g"][l].reshape(4, 128).T
        pblob[l, :, PB_LN1B:PB_LN1B + 4] = I["ln1_b"][l].reshape(4, 128).T
        pblob[l, :, PB_LN2G:PB_LN2G + 4] = I["ln2_g"][l].reshape(4, 128).T
        pblob[l, :, PB_LN2B:PB_LN2B + 4] = I["ln2_b"][l].reshape(4, 128).T
    vbias = np.ascontiguousarray(qkv_b[:, 2 * D:3 * D]).astype(BF)  # [LAYERS, D]
    fbrow = ff1_b.astype(BF)                             # [LAYERS, DFF]

    # attention bias+mask table over global (key, query) pairs
    i = np.arange(L)
    ni = np.clip(i - K // 2, 0, L - K)                   # [L] per query
    k_idx = np.arange(L)[:, None]                        # keys
    in_win = (k_idx >= ni[None, :]) & (k_idx < (ni + K)[None, :])   # [L keys, L q]
    rel = np.clip(k_idx - i[None, :] + (K - 1), 0, 2 * K - 2)       # [L, L]
    # B_full[l, h, k, q]
    rpb = I["rpb"]                                       # [LAYERS, H, 2K-1]
    B_full = np.where(in_win[None, None], rpb[:, :, rel], np.float32(NEG)).astype(BF)

    w_in_T = np.ascontiguousarray(I["in_w"].T).astype(BF)          # [PATCH, D]
    inb = np.ascontiguousarray(I["in_b"].reshape(DT, 128).T)       # [128, DT]
    out_wT = np.ascontiguousarray(I["out_w"].T)                    # [D, PATCH]
    w_out = part_major(out_wT).astype(BF)                          # [128, 4*PATCH]
    outb = np.ascontiguousarray(I["out_b"].reshape(PATCH, 1))

    x_tok = I["x"].reshape(L, PATCH)                     # [L, PATCH]

    in_maps = []
    for c in range(NC):
        xT_c = np.ascontiguousarray(x_tok[c * LC:(c + 1) * LC].T).astype(BF)
        r0 = min(max(c - 2, 0), 4)
        bmask_c = np.ascontiguousarray(
            B_full[:, :, 64 * r0:64 * r0 + WKEYS, c * LC:(c + 1) * LC])
        in_maps.append({
            "xT": xT_c,
            "w_in_T": w_in_T,
            "inb": inb,
            "wblob": wblob,
            "pblob": pblob,
            "vbias": vbias,
            "fbrow": fbrow,
            "bmask": bmask_c,
            "w_out": w_out,
            "outb": outb,
        })
    return in_maps


def kernel(**inputs):
    if "nc" not in _BUILD_CACHE:
        _BUILD_CACHE["nc"] = _build()
    nc = _BUILD_CACHE["nc"]
    in_maps = _prep_inputs(inputs)
    res = run_bass_kernel_spmd(nc, in_maps, core_ids=list(range(NC)))
    y = np.empty((1, 1, L * PATCH), np.float32)
    for c in range(NC):
        yT_c = res.results[c]["yT"]                      # [PATCH, LC]
        y[0, 0, c * LC * PATCH:(c + 1) * LC * PATCH] = yT_c.T.reshape(-1)
    return y



# revision 18
# speedup vs baseline: 1.0923x; 1.0923x over previous
"""Trainium2 Bass kernel for nn_AudioTransformer (neighborhood-attention transformer).

Strategy: sequence-parallel over 8 NeuronCores (64 tokens/core). Weights are
replicated per core in bf16 and streamed layer-by-layer (double-buffered,
issue split across engine DGE queues so no single queue serializes).
Activations live feature-major (features on SBUF partitions, tokens on the
free dim) so the whole layer stack runs without a single on-chip transpose.

Cross-core halo exchange per layer is a ReduceScatter: each core deposits its
LN1 output slab into the input slots of exactly the two neighbor cores that
need it (zeros elsewhere, pre-zeroed once at startup); the reduction collapses
the redundancy so the collective output is just the 2 halo slabs (128 tokens)
instead of a full 512-token AllGather — 18.3us vs 28.1us per layer.

The neighborhood attention runs dense over a 192-key window (ranks c-1,c,c+1)
with a host-precomputed transposed bias table (rel-pos bias inside the clamped
window, -60 outside) that is accumulated into the logits by the PE via an
identity matmul. Keys-on-partitions makes the softmax key-reduction a
ones-matmul; softmax skips max-subtraction (logits provably in [-2, 2]).
Q/K/V for the core's own tokens are computed during the collective.
"""

import numpy as np
import ml_dtypes

import concourse.bass as bass
import concourse.mybir as mybir
import concourse.tile as tile
from concourse import bacc
from concourse.bass_utils import run_bass_kernel_spmd


def _install_act_table_filter():
    """Make the act-table chooser resolve Ln/Exp/Identity/Copy only via the
    natural_log_exp_and_others set so each layer needs just 2 LUT swaps
    (to gelu_and_others and back) instead of 5. Positional set ids are
    preserved; sets are only shrunk, so every emitted load is still valid."""
    import concourse.bacc as _bacc_mod
    if getattr(_bacc_mod, "_ant_act_filter", False):
        return
    _orig = _bacc_mod.get_activation_tables
    A = mybir.ActivationFunctionType
    movable = {A.Ln, A.Exp, A.Identity, A.Copy}

    def _filtered(arch):
        t = _orig(arch)
        out = {}
        for name, funcs in t.items():
            if name == "natural_log_exp_and_others":
                out[name] = set(funcs)
            else:
                out[name] = set(funcs) - movable
        return out

    _bacc_mod.get_activation_tables = _filtered
    _bacc_mod._ant_act_filter = True

BF = ml_dtypes.bfloat16
F32 = mybir.dt.float32
BF16 = mybir.dt.bfloat16

NC = 8          # cores
L = 512         # total tokens
LC = L // NC    # tokens per core = 64
D = 512         # model dim
DT = D // 128   # 4 feature tiles
H = 8           # heads
DH = 64         # head dim
DFF = 2048      # ff dim
FT = DFF // 128  # 16 ff tiles
PATCH = 32
LAYERS = 8
K = 127         # neighborhood size
KW = 192        # per-core key window: ranks c-1, c, c+1 (64 each)
NEG = -60.0     # out-of-window logit bias (exp(-60+2) == 0 in fp32/bf16)
SLAB = D * LC   # one x~ slab, elements

# wblob column offsets (per 128-row partition, bf16)
OFF_QKV = 0            # 4 fi-tiles x 1536
OFF_PROJ = 6144        # 4 fi-tiles x 512
OFF_FF1 = 8192         # 4 fi-tiles x 2048
OFF_FF2 = 16384        # 16 fi-tiles x 512
WCOLS = 24576

# pblob columns (f32)
PB_QKVB = 0    # 12 (q 4, k 4, v 4 -- v unused on device, folded into proj_b)
PB_PROJB = 12  # 4
PB_FF1B = 16   # 16
PB_FF2B = 32   # 4
PCOLS = 36

_BUILD_CACHE = {}


def _build():
    """Build + finalize the SPMD Bass graph (same graph on all 8 cores)."""
    _install_act_table_filter()
    nc = bacc.Bacc(None, target_bir_lowering=False)
    AF = mybir.ActivationFunctionType

    # ---- DRAM parameters (per-core inputs) ----
    xT = nc.dram_tensor("xT", [PATCH, LC], BF16, kind="ExternalInput")
    w_in_T = nc.dram_tensor("w_in_T", [PATCH, D], BF16, kind="ExternalInput")
    inb = nc.dram_tensor("inb", [128, DT], F32, kind="ExternalInput")
    wblob = nc.dram_tensor("wblob", [LAYERS, 128, WCOLS], BF16, kind="ExternalInput")
    pblob = nc.dram_tensor("pblob", [LAYERS, 128, PCOLS], F32, kind="ExternalInput")
    bmt = nc.dram_tensor("bmt", [LAYERS, DH, H * KW], BF16, kind="ExternalInput")
    ident = nc.dram_tensor("ident", [DH, DH], BF16, kind="ExternalInput")
    w_out = nc.dram_tensor("w_out", [128, 128], BF16, kind="ExternalInput")
    outb = nc.dram_tensor("outb", [PATCH, 1], F32, kind="ExternalInput")
    yT = nc.dram_tensor("yT", [PATCH, LC], F32, kind="ExternalOutput")

    with tile.TileContext(nc) as tc:
        with (
            tc.tile_pool(name="singles", bufs=1) as singles,
            tc.tile_pool(name="wpool", bufs=2) as wpool,
            tc.tile_pool(name="ppool", bufs=2) as ppool,
            tc.tile_pool(name="bmpool", bufs=2) as bmpool,
            tc.tile_pool(name="actpool", bufs=2) as actpool,
            tc.tile_pool(name="gatherpool", bufs=2) as gatherpool,
            tc.tile_pool(name="tmppool", bufs=3) as tmppool,
            tc.tile_pool(name="statpool", bufs=4) as statpool,
            tc.tile_pool(name="agdram", bufs=2, space="DRAM") as agdram,
            tc.tile_pool(name="agdram1", bufs=1, space="DRAM") as agdram1,
            # PSUM: 8 banks total, every tile slot rounds to one bank.
            # pp:mm_out(3) + pp_ln:sums(1) + ppv(1) + ppatt:ps_l(2) + ppbc(1) = 8
            tc.tile_pool(name="pp", bufs=3, space="PSUM") as pp,
            tc.tile_pool(name="pp_ln", bufs=1, space="PSUM") as pp_ln,
            tc.tile_pool(name="ppv", bufs=1, space="PSUM") as ppv,
            tc.tile_pool(name="ppatt", bufs=2, space="PSUM") as ppatt,
            tc.tile_pool(name="ppbc", bufs=1, space="PSUM") as ppbc,
        ):
            # persistent tiles
            hT = singles.tile([128, DT, LC], F32)          # residual stream h.T
            ones_f = singles.tile([128, 1], F32)
            ones_b = singles.tile([128, 1], BF16)
            ones_row = singles.tile([1, 128], BF16)
            ones_bcf = singles.tile([1, 128], F32)
            xin = singles.tile([PATCH, LC], BF16)
            win = singles.tile([PATCH, D], BF16)
            inb_s = singles.tile([128, DT], F32)
            wout_s = singles.tile([128, 128], BF16)
            outb_s = singles.tile([PATCH, 1], F32)
            ident_s = singles.tile([DH, DH], BF16)
            zsb = singles.tile([128, 16, 256], BF16)       # zero fill source
            junk = singles.tile([1, 1], F32)               # act-table preload out

            nc.vector.memset(ones_f[:], 1.0)
            nc.vector.memset(ones_b[:], 1.0)
            nc.vector.memset(ones_row[:], 1.0)
            nc.vector.memset(ones_bcf[:], 1.0)
            nc.vector.memset(zsb[:], 0.0)
            nc.sync.dma_start(xin[:], xT[:])
            nc.sync.dma_start(win[:], w_in_T[:])
            nc.sync.dma_start(inb_s[:], inb[:])
            nc.scalar.dma_start(wout_s[:], w_out[:])
            nc.scalar.dma_start(outb_s[:], outb[:])
            nc.scalar.dma_start(ident_s[:], ident[:])

            # two persistent RS input buffers (one per layer parity), zeroed
            # once at startup; only the two per-layer halo slabs are ever
            # rewritten, so the zero slots stay zero for the whole run.
            rs_bufs = []
            for i, q in enumerate((nc.gpsimd, nc.scalar)):
                rs_b = agdram1.tile([16, 128, 256], BF16, tag=f"rs_in{i}",
                                    name="rs_b")
                q.dma_start(rs_b[:].rearrange("c p t -> p c t"), zsb[:])
                rs_bufs.append(rs_b)

            def layernorm(src, gcol, dst):
                """src [128,DT,LC] f32 -> dst [128,DT,LC] bf16 (normalized;
                gamma/beta folded into consumer matmul weights on host)."""
                ps_s = pp_ln.tile([1, 2 * LC], F32, tag="sums", name="ps_s")
                for f in range(DT):
                    nc.tensor.matmul(ps_s[0:1, 0:LC], ones_f[:], src[:, f, :],
                                     start=(f == 0), stop=(f == DT - 1))
                sq = tmppool.tile([128, DT, LC], F32, tag="ln_sq")
                nc.vector.tensor_mul(sq[:], src[:], src[:])
                for f in range(DT):
                    nc.tensor.matmul(ps_s[0:1, LC:2 * LC], ones_f[:], sq[:, f, :],
                                     start=(f == 0), stop=(f == DT - 1))
                st = statpool.tile([1, 2 * LC], F32, tag="ln_st")
                nc.vector.tensor_scalar_mul(st[0:1, 0:LC], ps_s[0:1, 0:LC], 1.0 / D)
                # (m2 - eps) elementwise, then var+eps = sumsq/D - (m2 - eps)
                m2 = statpool.tile([1, LC], F32, tag="ln_m2")
                nc.vector.tensor_mul(m2[:], st[0:1, 0:LC], st[0:1, 0:LC])
                nc.vector.tensor_scalar_add(m2[:], m2[:], -1e-5)
                var = statpool.tile([1, LC], F32, tag="ln_var")
                nc.vector.scalar_tensor_tensor(
                    var[:], ps_s[0:1, LC:2 * LC], 1.0 / D, m2[:],
                    op0=mybir.AluOpType.mult, op1=mybir.AluOpType.subtract)
                # rstd = exp(-0.5*ln(var)) -- keeps ACT in the Ln/Exp func set
                sd = statpool.tile([1, LC], F32, tag="ln_sd")
                nc.scalar.activation(sd[:], var[:], AF.Ln)
                nc.scalar.activation(st[0:1, LC:2 * LC], sd[:], AF.Exp, scale=-0.5)
                # broadcast (mean, rstd) across all 128 partitions via K=1 matmul
                bc = ppbc.tile([128, 2 * LC], F32, tag="bcast", name="bc")
                nc.tensor.matmul(bc[:], ones_bcf[:], st[:], start=True, stop=True)
                t0 = tmppool.tile([128, DT, LC], F32, tag="ln_t0")
                mean_w = bc[:, 0:LC].unsqueeze(1).to_broadcast([128, DT, LC])
                rstd_w = bc[:, LC:2 * LC].unsqueeze(1).to_broadcast([128, DT, LC])
                nc.vector.tensor_sub(t0[:], src[:], mean_w)
                nc.vector.tensor_mul(dst[:], t0[:], rstd_w)

            # ---- input projection: h0.T = in_w @ x_slice.T + in_b ----
            for t in range(DT):
                ps = pp.tile([128, LC], F32, tag="mm_out")
                nc.tensor.matmul(ps[:], win[:, t * 128:(t + 1) * 128], xin[:],
                                 start=True, stop=True)
                nc.scalar.activation(hT[:, t, :], ps[:], AF.Identity,
                                     bias=inb_s[:, t:t + 1], scale=1.0)

            # RS input slot chunks (chunk k = slot k//2, pos k%2):
            #   right-send -> (rank+1, pos0), rank 7 redirects to (0, pos0)
            #   left-send  -> (rank-1, pos1), rank 0 redirects to (7, pos1)
            # both redirect targets are halo positions nobody reads.
            rank = nc.sync.partition_id()
            ch_right = 2 * ((rank + 1) * (rank <= 6))
            rank_g = nc.gpsimd.partition_id()
            ch_left = 2 * ((rank_g - 1) * (rank_g >= 1) + 7 * (rank_g <= 0)) + 1

            def load_layer(l, queues):
                """Stream layer l's params; issue split across DGE queues."""
                w_qkv = wpool.tile([128, 6144], BF16, tag="w_qkv", name="w_qkv")
                w_proj = wpool.tile([128, 2048], BF16, tag="w_proj", name="w_proj")
                w_ff1 = wpool.tile([128, 8192], BF16, tag="w_ff1", name="w_ff1",
                                   bufs=3)
                w_ff2 = wpool.tile([128, 8192], BF16, tag="w_ff2", name="w_ff2",
                                   bufs=3)
                pb = ppool.tile([128, PCOLS], F32, tag="pb", name="pb")
                bm = bmpool.tile([DH, H * KW], BF16, tag="bm", name="bm")
                qs, qf1, qf2 = queues
                qs.dma_start(w_qkv[:], wblob[l, :, OFF_QKV:OFF_PROJ])
                qs.dma_start(w_proj[:], wblob[l, :, OFF_PROJ:OFF_FF1])
                qf1.dma_start(w_ff1[:], wblob[l, :, OFF_FF1:OFF_FF2])
                qf2.dma_start(w_ff2[:], wblob[l, :, OFF_FF2:WCOLS])
                qs.dma_start(pb[:], pblob[l])
                qf2.dma_start(bm[:], bmt[l])
                return w_qkv, w_proj, w_ff1, w_ff2, pb, bm

            cur = load_layer(0, (nc.sync, nc.sync, nc.scalar))
            for l in range(LAYERS):
                w_qkv, w_proj, w_ff1, w_ff2, pb, bm = cur

                # ---- LN1 ----
                xb = actpool.tile([128, DT, LC], BF16, tag="xb")
                layernorm(hT, PB_QKVB, xb)

                # ---- halo slab deposit + ReduceScatter ----
                rs_in = rs_bufs[l % 2]
                nc.sync.dma_start(
                    rs_in[bass.ds(ch_right, 1)]
                    .rearrange("c p (f t) -> p (c f) t", f=DT), xb[:])
                nc.gpsimd.dma_start(
                    rs_in[bass.ds(ch_left, 1)]
                    .rearrange("c p (f t) -> p (c f) t", f=DT), xb[:])
                rs_out = agdram.tile([2, SLAB], BF16, tag="rs_out",
                                     name="rs_out")
                with nc.allow_low_precision(
                        reason="exact: every summed region has a single "
                               "nonzero contributor, the rest are zeros"):
                    nc.gpsimd.collective_compute(
                        "ReduceScatter", mybir.AluOpType.add,
                        ins=[rs_in[:].opt()], outs=[rs_out[:].opt()],
                        replica_groups=[list(range(NC))])

                # prefetch next layer's weights NOW: the transfers stream
                # during the collective on idle queues.
                if l + 1 < LAYERS:
                    cur = load_layer(l + 1, (nc.sync, nc.sync, nc.scalar))

                # ---- during-RS compute: Q, own-K, own-V (local xb only) ----
                qT = []
                for t in range(DT):
                    ps = pp.tile([128, LC], F32, tag="mm_out")
                    for f in range(DT):
                        nc.tensor.matmul(
                            ps[:],
                            w_qkv[:, f * 1536 + t * 128:f * 1536 + (t + 1) * 128],
                            xb[:, f, :], start=(f == 0), stop=(f == DT - 1))
                    qT_t = actpool.tile([128, LC], BF16, tag=f"qT{t}", name="qT_t")
                    nc.scalar.activation(qT_t[:], ps[:], AF.Identity,
                                         bias=pb[:, PB_QKVB + t:PB_QKVB + t + 1],
                                         scale=1.0)
                    qT.append(qT_t)

                ps_k = [ppatt.tile([128, 2, KW], F32, tag="ps_l", name="ps_k")
                        for _ in range(2)]
                KTg = [gatherpool.tile([128, KW], BF16, tag=f"KTg{g}",
                                       name="KTg_g") for g in range(DT)]
                for g in range(DT):
                    for f in range(DT):
                        nc.tensor.matmul(
                            ps_k[g // 2][:, g % 2, 64:128],
                            w_qkv[:, f * 1536 + 512 + g * 128:
                                  f * 1536 + 512 + (g + 1) * 128],
                            xb[:, f, :], start=(f == 0), stop=(f == DT - 1))
                    nc.scalar.activation(
                        KTg[g][:, 64:128], ps_k[g // 2][:, g % 2, 64:128],
                        AF.Identity,
                        bias=pb[:, PB_QKVB + DT + g:PB_QKVB + DT + g + 1],
                        scale=1.0)

                Vt0 = gatherpool.tile([128, D], BF16, tag="Vt0", name="Vt0")
                Vt1 = gatherpool.tile([DH, D], BF16, tag="Vt1", name="Vt1")
                ps_v = ppv.tile([128, D], F32, tag="ps_v")
                for f in range(DT):
                    nc.tensor.matmul(
                        ps_v[64:128, :], xb[:, f, :],
                        w_qkv[:, f * 1536 + 1024:f * 1536 + 1536],
                        start=(f == 0), stop=(f == DT - 1))
                nc.scalar.copy(Vt0[64:128, :], ps_v[64:128, :])

                # ---- gathered halo slabs -> SBUF ----
                xwh = gatherpool.tile([128, 2, DT, LC], BF16, tag="xwh",
                                      name="xwh")
                nc.sync.dma_start(
                    xwh[:],
                    rs_out[:].rearrange("s (p f t) -> p s f t", p=128, f=DT))

                # ---- halo K (window cols L=0:64, R=128:192) ----
                for g in range(DT):
                    for s, c0 in ((0, 0), (1, 128)):
                        for f in range(DT):
                            nc.tensor.matmul(
                                ps_k[g // 2][:, g % 2, c0:c0 + 64],
                                w_qkv[:, f * 1536 + 512 + g * 128:
                                      f * 1536 + 512 + (g + 1) * 128],
                                xwh[:, s, f, :], start=(f == 0),
                                stop=(f == DT - 1))
                    nc.scalar.activation(
                        KTg[g][:, 0:64], ps_k[g // 2][:, g % 2, 0:64],
                        AF.Identity,
                        bias=pb[:, PB_QKVB + DT + g:PB_QKVB + DT + g + 1],
                        scale=1.0)
                    nc.scalar.activation(
                        KTg[g][:, 128:192], ps_k[g // 2][:, g % 2, 128:192],
                        AF.Identity,
                        bias=pb[:, PB_QKVB + DT + g:PB_QKVB + DT + g + 1],
                        scale=1.0)

                # ---- halo V: L tokens -> Vt0[0:64], R tokens -> Vt1 ----
                for f in range(DT):
                    nc.tensor.matmul(
                        ps_v[0:64, :], xwh[:, 0, f, :],
                        w_qkv[:, f * 1536 + 1024:f * 1536 + 1536],
                        start=(f == 0), stop=(f == DT - 1))
                nc.scalar.copy(Vt0[0:64, :], ps_v[0:64, :])
                ps_v2 = ppv.tile([128, D], F32, tag="ps_v")
                for f in range(DT):
                    nc.tensor.matmul(
                        ps_v2[0:64, :], xwh[:, 1, f, :],
                        w_qkv[:, f * 1536 + 1024:f * 1536 + 1536],
                        start=(f == 0), stop=(f == DT - 1))
                nc.scalar.copy(Vt1[:], ps_v2[0:64, :])

                # ---- attention (per-head tiles so QK/softmax/AV pipeline) ----
                probs = []
                ps_sum = pp_ln.tile([1, H * LC], F32, tag="sums", name="ps_sum")
                for h in range(H):
                    hh, g = h % 2, h // 2
                    ps_l = ppatt.tile([128, 2, LC], F32, tag="ps_l", name="ps_l")
                    # kt0 = 128 keys (L+own); bias+mask accumulated by PE
                    nc.tensor.matmul(ps_l[:, 0, :],
                                     KTg[g][hh * DH:(hh + 1) * DH, 0:128],
                                     qT[g][hh * DH:(hh + 1) * DH, :],
                                     start=True, stop=False)
                    nc.tensor.matmul(ps_l[:, 0, :],
                                     bm[:, h * KW:h * KW + 128], ident_s[:],
                                     start=False, stop=True)
                    # kt1 = 64 keys (R), partitions 0:64
                    nc.tensor.matmul(ps_l[0:64, 1, :],
                                     KTg[g][hh * DH:(hh + 1) * DH, 128:192],
                                     qT[g][hh * DH:(hh + 1) * DH, :],
                                     start=True, stop=False)
                    nc.tensor.matmul(ps_l[0:64, 1, :],
                                     bm[:, h * KW + 128:h * KW + KW], ident_s[:],
                                     start=False, stop=True)
                    probs_h = actpool.tile([128, 2, LC], BF16, tag=f"probs{h}",
                                           name="probs_h")
                    nc.scalar.activation(probs_h[:, 0, :], ps_l[:, 0, :], AF.Exp)
                    nc.scalar.activation(probs_h[0:64, 1, :], ps_l[0:64, 1, :],
                                         AF.Exp)
                    probs.append(probs_h)
                    nc.tensor.matmul(ps_sum[0:1, h * LC:(h + 1) * LC],
                                     ones_b[:], probs_h[:, 0, :],
                                     start=True, stop=False)
                    nc.tensor.matmul(ps_sum[0:1, h * LC:(h + 1) * LC],
                                     ones_b[0:64, :], probs_h[0:64, 1, :],
                                     start=False, stop=True)
                # denominators -> bf16 reciprocal -> PE broadcast to DH rows
                rs_r = statpool.tile([1, H * LC], BF16, tag="rs_r")
                with nc.allow_low_precision(
                        reason="softmax 1/denom in bf16; error is ~0.4% of "
                               "the attention output, well inside tolerance"):
                    nc.vector.reciprocal(rs_r[:], ps_sum[:])
                rs_bc = ppbc.tile([DH, H * LC], F32, tag="bcast", name="rs_bc")
                nc.tensor.matmul(rs_bc[:], ones_row[0:1, 0:DH], rs_r[:],
                                 start=True, stop=True)
                # AV, one output tile per head-pair
                oT = [actpool.tile([128, LC], BF16, tag=f"oT{g}", name="oT_g")
                      for g in range(DT)]
                for h in range(H):
                    hh, g = h % 2, h // 2
                    ps_o = pp.tile([DH, LC], F32, tag="mm_out", name="ps_o")
                    nc.tensor.matmul(ps_o[:], Vt0[:, h * DH:(h + 1) * DH],
                                     probs[h][:, 0, :], start=True, stop=False)
                    nc.tensor.matmul(ps_o[:], Vt1[:, h * DH:(h + 1) * DH],
                                     probs[h][0:64, 1, :], start=False, stop=True)
                    nc.vector.tensor_mul(
                        oT[g][hh * DH:(hh + 1) * DH, :], ps_o[:],
                        rs_bc[:, h * LC:(h + 1) * LC])

                # ---- proj + residual ----
                for t in range(DT):
                    ps = pp.tile([128, LC], F32, tag="mm_out")
                    for f in range(DT):
                        nc.tensor.matmul(
                            ps[:],
                            w_proj[:, f * 512 + t * 128:f * 512 + (t + 1) * 128],
                            oT[f][:], start=(f == 0), stop=(f == DT - 1))
                    nc.vector.scalar_tensor_tensor(
                        hT[:, t, :], ps[:], pb[:, PB_PROJB + t:PB_PROJB + t + 1],
                        hT[:, t, :], op0=mybir.AluOpType.add,
                        op1=mybir.AluOpType.add)

                # ---- LN2 ----
                zb = actpool.tile([128, DT, LC], BF16, tag="zb")
                layernorm(hT, 0, zb)

                # preload the gelu act table while ACT is otherwise idle
                nc.scalar.activation(junk[:], ones_f[0:1, 0:1], AF.Gelu)

                # ---- FF1 + gelu (z1 split in two tiles so FF2 can start
                # accumulating after the first half) ----
                FH = FT // 2
                z1a = actpool.tile([128, FH, LC], BF16, tag="z1a")
                z1b = actpool.tile([128, FH, LC], BF16, tag="z1b")
                for tq in range(FT // 4):
                    ps = pp.tile([128, 4, LC], F32, tag="mm_out", name="ps_ff1")
                    z1d = z1a if tq < 2 else z1b
                    for tt in range(4):
                        t = tq * 4 + tt
                        for f in range(DT):
                            nc.tensor.matmul(
                                ps[:, tt, :],
                                w_ff1[:, f * 2048 + t * 128:
                                      f * 2048 + (t + 1) * 128],
                                zb[:, f, :], start=(f == 0), stop=(f == DT - 1))
                        nc.scalar.activation(
                            z1d[:, (tq % 2) * 4 + tt, :], ps[:, tt, :], AF.Gelu,
                            bias=pb[:, PB_FF1B + t:PB_FF1B + t + 1], scale=1.0)

                # switch the act table back while ACT idles through FF2
                if l + 1 < LAYERS:
                    nc.scalar.activation(junk[:], ones_f[0:1, 0:1], AF.Exp)

                # ---- FF2 + residual ----
                for t in range(DT):
                    ps = pp.tile([128, LC], F32, tag="mm_out")
                    for g in range(FT):
                        z1d = z1a if g < FH else z1b
                        nc.tensor.matmul(
                            ps[:],
                            w_ff2[:, g * 512 + t * 128:g * 512 + (t + 1) * 128],
                            z1d[:, g % FH, :], start=(g == 0), stop=(g == FT - 1))
                    nc.vector.scalar_tensor_tensor(
                        hT[:, t, :], ps[:], pb[:, PB_FF2B + t:PB_FF2B + t + 1],
                        hT[:, t, :], op0=mybir.AluOpType.add,
                        op1=mybir.AluOpType.add)

            # ---- output projection: y.T = tanh(out_w @ h.T + out_b) ----
            # (act table is gelu_and_others here, which contains Tanh)
            hb = actpool.tile([128, DT, LC], BF16, tag="hb")
            nc.vector.tensor_copy(hb[:], hT[:])
            ps_y = pp.tile([PATCH, LC], F32, tag="mm_out", name="ps_y")
            for f in range(DT):
                nc.tensor.matmul(ps_y[:], wout_s[:, f * PATCH:(f + 1) * PATCH],
                                 hb[:, f, :], start=(f == 0), stop=(f == DT - 1))
            y_sb = actpool.tile([PATCH, LC], F32, tag="y_sb")
            nc.scalar.activation(y_sb[:], ps_y[:], AF.Tanh,
                                 bias=outb_s[:, 0:1], scale=1.0)
            nc.sync.dma_start(yT[:], y_sb[:])

    nc.finalize()
    return nc


def _prep_inputs(inputs):
    """Host-side: pack full fp32 inputs into per-core in_maps."""
    I = {k: np.asarray(v, np.float32) for k, v in inputs.items()}

    scale = np.float32(DH ** -0.5)
    qkv_w = I["qkv_w"].copy()          # [LAYERS, 3D, D]
    qkv_b = I["qkv_b"].copy()          # [LAYERS, 3D]
    ff1_w = I["ff1_w"].copy()          # [LAYERS, DFF, D]
    ff1_b = I["ff1_b"].copy()          # [LAYERS, DFF]
    proj_b = I["proj_b"].copy()        # [LAYERS, D]
    # fold LN affines into the consuming matmuls (exact algebra, fp32):
    # (xn*g + b) @ W.T = xn @ (W*diag(g)).T + W@b
    for l in range(LAYERS):
        qkv_b[l] += qkv_w[l] @ I["ln1_b"][l]
        qkv_w[l] *= I["ln1_g"][l][None, :]
        ff1_b[l] += ff1_w[l] @ I["ln2_b"][l]
        ff1_w[l] *= I["ln2_g"][l][None, :]
        # v-bias passes through softmax-normalized attention exactly:
        # o = sum_k a_k (v_k + bv) = o' + bv  ->  fold W_o @ bv into proj_b
        proj_b[l] += I["proj_w"][l] @ qkv_b[l, 2 * D:]
    qkv_w[:, :D] *= scale
    qkv_b[:, :D] *= scale

    def part_major(m):
        # [X*128, Y] -> [128, X*Y] with column blocks per 128-row tile
        X = m.shape[0] // 128
        return np.ascontiguousarray(
            m.reshape(X, 128, m.shape[1]).transpose(1, 0, 2).reshape(128, -1))

    wblob = np.empty((LAYERS, 128, WCOLS), BF)
    pblob = np.empty((LAYERS, 128, PCOLS), np.float32)
    for l in range(LAYERS):
        qkvT = np.ascontiguousarray(qkv_w[l].T)          # [D, 3D]
        projT = np.ascontiguousarray(I["proj_w"][l].T)   # [D, D]
        ff1T = np.ascontiguousarray(ff1_w[l].T)          # [D, DFF]
        ff2T = np.ascontiguousarray(I["ff2_w"][l].T)     # [DFF, D]
        wblob[l, :, OFF_QKV:OFF_PROJ] = part_major(qkvT).astype(BF)
        wblob[l, :, OFF_PROJ:OFF_FF1] = part_major(projT).astype(BF)
        wblob[l, :, OFF_FF1:OFF_FF2] = part_major(ff1T).astype(BF)
        wblob[l, :, OFF_FF2:WCOLS] = part_major(ff2T).astype(BF)
        pblob[l, :, PB_QKVB:PB_QKVB + 12] = qkv_b[l].reshape(12, 128).T
        pblob[l, :, PB_PROJB:PB_PROJB + 4] = proj_b[l].reshape(4, 128).T
        pblob[l, :, PB_FF1B:PB_FF1B + 16] = ff1_b[l].reshape(16, 128).T
        pblob[l, :, PB_FF2B:PB_FF2B + 4] = I["ff2_b"][l].reshape(4, 128).T

    # transposed attention bias+mask table over the per-core 192-key window
    # bmt_c[l, q, h, wk] = rpb[l, h, (g-q)+K-1] inside the clamped window
    # else NEG, where g = 64*(c-1) + wk is the global key index.
    i = np.arange(L)
    ni = np.clip(i - K // 2, 0, L - K)                   # [L] window starts
    rpb = I["rpb"]                                       # [LAYERS, H, 2K-1]

    w_in_T = np.ascontiguousarray(I["in_w"].T).astype(BF)          # [PATCH, D]
    inb = np.ascontiguousarray(I["in_b"].reshape(DT, 128).T)       # [128, DT]
    out_wT = np.ascontiguousarray(I["out_w"].T)                    # [D, PATCH]
    w_out = part_major(out_wT).astype(BF)                          # [128, 4*PATCH]
    outb = np.ascontiguousarray(I["out_b"].reshape(PATCH, 1))
    ident = np.eye(DH, dtype=BF)

    x_tok = I["x"].reshape(L, PATCH)                     # [L, PATCH]

    in_maps = []
    for c in range(NC):
        xT_c = np.ascontiguousarray(x_tok[c * LC:(c + 1) * LC].T).astype(BF)
        q = c * LC + np.arange(LC)                       # [LC] global queries
        g = 64 * (c - 1) + np.arange(KW)                 # [KW] global keys
        in_win = ((g[None, :] >= 0) & (g[None, :] < L)
                  & (g[None, :] >= ni[q][:, None])
                  & (g[None, :] < (ni[q] + K)[:, None]))             # [LC, KW]
        rel = np.clip(g[None, :] - q[:, None] + (K - 1), 0, 2 * K - 2)
        # [LAYERS, H, LC, KW] -> transpose to [LAYERS, LC(q), H, KW]
        bmt_c = np.where(in_win[None, None], rpb[:, :, rel],
                         np.float32(NEG)).transpose(0, 2, 1, 3)
        bmt_c = np.ascontiguousarray(
            bmt_c.reshape(LAYERS, LC, H * KW)).astype(BF)
        in_maps.append({
            "xT": xT_c,
            "w_in_T": w_in_T,
            "inb": inb,
            "wblob": wblob,
            "pblob": pblob,
            "bmt": bmt_c,
            "ident": ident,
            "w_out": w_out,
            "outb": outb,
        })
    return in_maps


def kernel(**inputs):
    if "nc" not in _BUILD_CACHE:
        _BUILD_CACHE["nc"] = _build()
    nc = _BUILD_CACHE["nc"]
    in_maps = _prep_inputs(inputs)
    res = run_bass_kernel_spmd(nc, in_maps, core_ids=list(range(NC)))
    y = np.empty((1, 1, L * PATCH), np.float32)
    for c in range(NC):
        yT_c = res.results[c]["yT"]                      # [PATCH, LC]
        y[0, 0, c * LC * PATCH:(c + 1) * LC * PATCH] = yT_c.T.reshape(-1)
    return y


# revision 36
# speedup vs baseline: 1.1968x; 1.0958x over previous
"""Trainium2 Bass kernel for nn_AudioTransformer (neighborhood-attention transformer).

Strategy: sequence-parallel over 8 NeuronCores (64 tokens/core). Weights are
replicated per core in bf16 and streamed layer-by-layer (double-buffered,
issue split across engine DGE queues so no single queue serializes).
Activations live feature-major (features on SBUF partitions, tokens on the
free dim) so the whole layer stack runs without a single on-chip transpose.

Cross-core halo exchange per layer is a ReduceScatter: each core deposits its
LN1 output slab into the input slots of exactly the two neighbor cores that
need it (zeros elsewhere, pre-zeroed once at startup); the reduction collapses
the redundancy so the collective output is just the 2 halo slabs (128 tokens)
instead of a full 512-token AllGather — 18.3us vs 28.1us per layer.

The neighborhood attention runs dense over a 192-key window (ranks c-1,c,c+1)
with a host-precomputed transposed bias table (rel-pos bias inside the clamped
window, -60 outside) that is accumulated into the logits by the PE via an
identity matmul. Keys-on-partitions makes the softmax key-reduction a
ones-matmul; softmax skips max-subtraction (logits provably in [-2, 2]).
Q/K/V for the core's own tokens are computed during the collective.
"""

import numpy as np
import ml_dtypes

import concourse.bass as bass
import concourse.mybir as mybir
import concourse.tile as tile
from concourse.tile import add_dep_helper
from concourse import bacc
from concourse.bass_utils import run_bass_kernel_spmd


def _install_act_table_filter():
    """Make the act-table chooser resolve Ln/Exp/Identity/Copy only via the
    natural_log_exp_and_others set so each layer needs just 2 LUT swaps
    (to gelu_and_others and back) instead of 5. Positional set ids are
    preserved; sets are only shrunk, so every emitted load is still valid."""
    import concourse.bacc as _bacc_mod
    if getattr(_bacc_mod, "_ant_act_filter", False):
        return
    _orig = _bacc_mod.get_activation_tables
    A = mybir.ActivationFunctionType
    movable = {A.Ln, A.Exp, A.Identity, A.Copy}

    def _filtered(arch):
        t = _orig(arch)
        out = {}
        for name, funcs in t.items():
            if name == "natural_log_exp_and_others":
                out[name] = set(funcs)
            else:
                out[name] = set(funcs) - movable
        return out

    _bacc_mod.get_activation_tables = _filtered
    _bacc_mod._ant_act_filter = True

BF = ml_dtypes.bfloat16
F32 = mybir.dt.float32
BF16 = mybir.dt.bfloat16

NC = 8          # cores
L = 512         # total tokens
LC = L // NC    # tokens per core = 64
D = 512         # model dim
DT = D // 128   # 4 feature tiles
H = 8           # heads
DH = 64         # head dim
DFF = 2048      # ff dim
FT = DFF // 128  # 16 ff tiles
PATCH = 32
LAYERS = 8
K = 127         # neighborhood size
KW = 192        # per-core key window: ranks c-1, c, c+1 (64 each)
NEG = -60.0     # out-of-window logit bias (exp(-60+2) == 0 in fp32/bf16)
SLAB = D * LC   # one x~ slab, elements

# wblob column offsets (per 128-row partition, bf16)
OFF_QKV = 0            # 4 fi-tiles x 1536
OFF_PROJ = 6144        # 4 fi-tiles x 512
OFF_FF1 = 8192         # 4 fi-tiles x 2048
OFF_FF2 = 16384        # 16 fi-tiles x 512
WCOLS = 24576

# pblob columns (f32)
PB_QKVB = 0    # 12 (q 4, k 4, v 4 -- v unused on device, folded into proj_b)
PB_PROJB = 12  # 4
PB_FF1B = 16   # 16
PB_FF2B = 32   # 4
PCOLS = 36

_BUILD_CACHE = {}


def _build():
    """Build + finalize the SPMD Bass graph (same graph on all 8 cores)."""
    _install_act_table_filter()
    nc = bacc.Bacc(None, target_bir_lowering=False)
    AF = mybir.ActivationFunctionType

    # ---- DRAM parameters (per-core inputs) ----
    xT = nc.dram_tensor("xT", [PATCH, LC], BF16, kind="ExternalInput")
    w_in_T = nc.dram_tensor("w_in_T", [PATCH, D], BF16, kind="ExternalInput")
    inb = nc.dram_tensor("inb", [128, DT], F32, kind="ExternalInput")
    wblob = nc.dram_tensor("wblob", [LAYERS, 128, WCOLS], BF16, kind="ExternalInput")
    pblob = nc.dram_tensor("pblob", [LAYERS, 128, PCOLS], F32, kind="ExternalInput")
    fbrow = nc.dram_tensor("fbrow", [LAYERS, DFF], BF16, kind="ExternalInput")
    bmt = nc.dram_tensor("bmt", [LAYERS, DH, H * KW], BF16, kind="ExternalInput")
    ident = nc.dram_tensor("ident", [DH, DH], BF16, kind="ExternalInput")
    w_out = nc.dram_tensor("w_out", [128, 128], BF16, kind="ExternalInput")
    outb = nc.dram_tensor("outb", [PATCH, 1], F32, kind="ExternalInput")
    yT = nc.dram_tensor("yT", [PATCH, LC], F32, kind="ExternalOutput")

    with tile.TileContext(nc) as tc:
        with (
            tc.tile_pool(name="singles", bufs=1) as singles,
            tc.tile_pool(name="wpool", bufs=2) as wpool,
            tc.tile_pool(name="ppool", bufs=2) as ppool,
            tc.tile_pool(name="bmpool", bufs=2) as bmpool,
            tc.tile_pool(name="actpool", bufs=2) as actpool,
            tc.tile_pool(name="gatherpool", bufs=2) as gatherpool,
            tc.tile_pool(name="tmppool", bufs=3) as tmppool,
            tc.tile_pool(name="statpool", bufs=4) as statpool,
            tc.tile_pool(name="agdram", bufs=2, space="DRAM") as agdram,
            tc.tile_pool(name="agdram1", bufs=1, space="DRAM") as agdram1,
            # PSUM: 8 banks total, every tile slot rounds to one bank.
            # pp:mm_out(3) + pp_ln:sums(1) + ppv(1) + ppatt:ps_l(2) + ppbc(1) = 8
            tc.tile_pool(name="pp", bufs=3, space="PSUM") as pp,
            tc.tile_pool(name="pp_ln", bufs=1, space="PSUM") as pp_ln,
            tc.tile_pool(name="ppv", bufs=1, space="PSUM") as ppv,
            tc.tile_pool(name="ppatt", bufs=2, space="PSUM") as ppatt,
            tc.tile_pool(name="ppbc", bufs=1, space="PSUM") as ppbc,
        ):
            # persistent tiles
            hT = singles.tile([128, DT, LC], F32)          # residual stream h.T
            ones_f = singles.tile([128, 1], F32)
            ones_b = singles.tile([128, 1], BF16)
            ones_row = singles.tile([1, 128], BF16)
            ones_bcf = singles.tile([1, 128], F32)
            xin = singles.tile([PATCH, LC], BF16)
            win = singles.tile([PATCH, D], BF16)
            inb_s = singles.tile([128, DT], F32)
            wout_s = singles.tile([128, 128], BF16)
            outb_s = singles.tile([PATCH, 1], F32)
            ident_s = singles.tile([DH, DH], BF16)
            zsb = singles.tile([128, 16, 256], BF16)       # zero fill source
            junk = singles.tile([1, 1], F32)               # act-table preload out

            nc.vector.memset(ones_f[:], 1.0)
            nc.vector.memset(ones_b[:], 1.0)
            nc.vector.memset(ones_row[:], 1.0)
            nc.vector.memset(ones_bcf[:], 1.0)
            nc.vector.memset(zsb[:], 0.0)
            nc.sync.dma_start(xin[:], xT[:])
            nc.sync.dma_start(win[:], w_in_T[:])
            nc.sync.dma_start(inb_s[:], inb[:])
            nc.scalar.dma_start(wout_s[:], w_out[:])
            nc.scalar.dma_start(outb_s[:], outb[:])
            nc.scalar.dma_start(ident_s[:], ident[:])

            # two persistent RS input buffers (one per layer parity), zeroed
            # once at startup; only the two per-layer halo slabs are ever
            # rewritten, so the zero slots stay zero for the whole run.
            rs_bufs = []
            for i, q in enumerate((nc.gpsimd, nc.scalar)):
                rs_b = agdram1.tile([16, 128, 256], BF16, tag=f"rs_in{i}",
                                    name="rs_b")
                q.dma_start(rs_b[:].rearrange("c p t -> p c t"), zsb[:])
                rs_bufs.append(rs_b)

            def layernorm(src, gcol, dst):
                """src [128,DT,LC] f32 -> dst [128,DT,LC] bf16 (normalized;
                gamma/beta folded into consumer matmul weights on host).
                Returns the final ACT (Exp) instruction for dep pinning."""
                ps_s = pp_ln.tile([1, 2 * LC], F32, tag="sums", name="ps_s")
                for f in range(DT):
                    nc.tensor.matmul(ps_s[0:1, 0:LC], ones_f[:], src[:, f, :],
                                     start=(f == 0), stop=(f == DT - 1))
                sq = tmppool.tile([128, DT, LC], F32, tag="ln_sq")
                nc.vector.tensor_mul(sq[:], src[:], src[:])
                for f in range(DT):
                    nc.tensor.matmul(ps_s[0:1, LC:2 * LC], ones_f[:], sq[:, f, :],
                                     start=(f == 0), stop=(f == DT - 1))
                st = statpool.tile([1, 2 * LC], F32, tag="ln_st")
                nc.vector.tensor_scalar_mul(st[0:1, 0:LC], ps_s[0:1, 0:LC], 1.0 / D)
                # (m2 - eps) elementwise, then var+eps = sumsq/D - (m2 - eps)
                m2 = statpool.tile([1, LC], F32, tag="ln_m2")
                nc.vector.tensor_mul(m2[:], st[0:1, 0:LC], st[0:1, 0:LC])
                nc.vector.tensor_scalar_add(m2[:], m2[:], -1e-5)
                var = statpool.tile([1, LC], F32, tag="ln_var")
                nc.vector.scalar_tensor_tensor(
                    var[:], ps_s[0:1, LC:2 * LC], 1.0 / D, m2[:],
                    op0=mybir.AluOpType.mult, op1=mybir.AluOpType.subtract)
                # rstd = exp(-0.5*ln(var)) -- keeps ACT in the Ln/Exp func set
                sd = statpool.tile([1, LC], F32, tag="ln_sd")
                nc.scalar.activation(sd[:], var[:], AF.Ln)
                exp_i = nc.scalar.activation(st[0:1, LC:2 * LC], sd[:], AF.Exp,
                                             scale=-0.5)
                # broadcast (mean, rstd) across all 128 partitions via K=1 matmul
                bc = ppbc.tile([128, 2 * LC], F32, tag="bcast", name="bc")
                nc.tensor.matmul(bc[:], ones_bcf[:], st[:], start=True, stop=True)
                t0 = tmppool.tile([128, DT, LC], F32, tag="ln_t0")
                mean_w = bc[:, 0:LC].unsqueeze(1).to_broadcast([128, DT, LC])
                rstd_w = bc[:, LC:2 * LC].unsqueeze(1).to_broadcast([128, DT, LC])
                nc.vector.tensor_sub(t0[:], src[:], mean_w)
                nc.vector.tensor_mul(dst[:], t0[:], rstd_w)
                return exp_i

            # ---- input projection: h0.T = in_w @ x_slice.T + in_b ----
            for t in range(DT):
                ps = pp.tile([128, LC], F32, tag="mm_out")
                nc.tensor.matmul(ps[:], win[:, t * 128:(t + 1) * 128], xin[:],
                                 start=True, stop=True)
                nc.scalar.activation(hT[:, t, :], ps[:], AF.Identity,
                                     bias=inb_s[:, t:t + 1], scale=1.0)

            # RS input slot chunks (chunk k = slot k//2, pos k%2):
            #   right-send -> (rank+1, pos0), rank 7 redirects to (0, pos0)
            #   left-send  -> (rank-1, pos1), rank 0 redirects to (7, pos1)
            # both redirect targets are halo positions nobody reads.
            rank = nc.sync.partition_id()
            ch_right = 2 * ((rank + 1) * (rank <= 6))
            rank_g = nc.gpsimd.partition_id()
            ch_left = 2 * ((rank_g - 1) * (rank_g >= 1) + 7 * (rank_g <= 0)) + 1

            def load_layer(l, queues):
                """Stream layer l's params; issue split across DGE queues."""
                w_qkv = wpool.tile([128, 6144], BF16, tag="w_qkv", name="w_qkv")
                w_proj = wpool.tile([128, 2048], BF16, tag="w_proj", name="w_proj")
                w_ff1 = wpool.tile([128, 8192], BF16, tag="w_ff1", name="w_ff1",
                                   bufs=3)
                w_ff2 = wpool.tile([128, 8192], BF16, tag="w_ff2", name="w_ff2",
                                   bufs=3)
                pb = ppool.tile([128, PCOLS], F32, tag="pb", name="pb")
                fb = ppool.tile([1, DFF], BF16, tag="fb", name="fb")
                bm = bmpool.tile([DH, H * KW], BF16, tag="bm", name="bm")
                qs, qp = queues
                qs.dma_start(w_qkv[:], wblob[l, :, OFF_QKV:OFF_PROJ])
                qs.dma_start(w_proj[:], wblob[l, :, OFF_PROJ:OFF_FF1])
                qs.dma_start(w_ff1[:], wblob[l, :, OFF_FF1:OFF_FF2])
                qs.dma_start(pb[:], pblob[l])
                qp.dma_start(w_ff2[:], wblob[l, :, OFF_FF2:WCOLS])
                qp.dma_start(fb[:], fbrow[l].unsqueeze(0))
                qp.dma_start(bm[:], bmt[l])
                return w_qkv, w_proj, w_ff1, w_ff2, pb, fb, bm

            cur = load_layer(0, (nc.sync, nc.gpsimd))
            for l in range(LAYERS):
                w_qkv, w_proj, w_ff1, w_ff2, pb, fb, bm = cur

                # ---- LN1 ----
                xb = actpool.tile([128, DT, LC], BF16, tag="xb")
                layernorm(hT, PB_QKVB, xb)

                # ---- halo slab deposit + ReduceScatter ----
                rs_in = rs_bufs[l % 2]
                nc.sync.dma_start(
                    rs_in[bass.ds(ch_right, 1)]
                    .rearrange("c p (f t) -> p (c f) t", f=DT), xb[:])
                nc.gpsimd.dma_start(
                    rs_in[bass.ds(ch_left, 1)]
                    .rearrange("c p (f t) -> p (c f) t", f=DT), xb[:])
                rs_out = agdram.tile([2, SLAB], BF16, tag="rs_out",
                                     name="rs_out")
                with nc.allow_low_precision(
                        reason="exact: every summed region has a single "
                               "nonzero contributor, the rest are zeros"):
                    nc.gpsimd.collective_compute(
                        "ReduceScatter", mybir.AluOpType.add,
                        ins=[rs_in[:].opt()], outs=[rs_out[:].opt()],
                        replica_groups=[list(range(NC))])

                # prefetch next layer's weights NOW: the transfers stream
                # during the collective on idle queues.
                if l + 1 < LAYERS:
                    cur = load_layer(l + 1, (nc.sync, nc.gpsimd))

                # ---- during-RS compute: Q, own-K, own-V (local xb only) ----
                qT = []
                for t in range(DT):
                    ps = pp.tile([128, LC], F32, tag="mm_out")
                    for f in range(DT):
                        nc.tensor.matmul(
                            ps[:],
                            w_qkv[:, f * 1536 + t * 128:f * 1536 + (t + 1) * 128],
                            xb[:, f, :], start=(f == 0), stop=(f == DT - 1))
                    qT_t = actpool.tile([128, LC], BF16, tag=f"qT{t}", name="qT_t")
                    nc.vector.tensor_scalar_add(
                        qT_t[:], ps[:], pb[:, PB_QKVB + t:PB_QKVB + t + 1])
                    qT.append(qT_t)

                ps_k = [ppatt.tile([128, 2, KW], F32, tag="ps_l", name="ps_k")
                        for _ in range(2)]
                KTg = [gatherpool.tile([128, KW], BF16, tag=f"KTg{g}",
                                       name="KTg_g") for g in range(DT)]
                for g in range(DT):
                    for f in range(DT):
                        nc.tensor.matmul(
                            ps_k[g // 2][:, g % 2, 64:128],
                            w_qkv[:, f * 1536 + 512 + g * 128:
                                  f * 1536 + 512 + (g + 1) * 128],
                            xb[:, f, :], start=(f == 0), stop=(f == DT - 1))
                    nc.vector.tensor_scalar_add(
                        KTg[g][:, 64:128], ps_k[g // 2][:, g % 2, 64:128],
                        pb[:, PB_QKVB + DT + g:PB_QKVB + DT + g + 1])

                Vt0 = gatherpool.tile([128, D], BF16, tag="Vt0", name="Vt0")
                Vt1 = gatherpool.tile([DH, D], BF16, tag="Vt1", name="Vt1")
                ps_v = ppv.tile([128, D], F32, tag="ps_v")
                for f in range(DT):
                    nc.tensor.matmul(
                        ps_v[64:128, :], xb[:, f, :],
                        w_qkv[:, f * 1536 + 1024:f * 1536 + 1536],
                        start=(f == 0), stop=(f == DT - 1))
                nc.vector.tensor_copy(Vt0[64:128, :], ps_v[64:128, :])

                # ---- gathered halo slabs -> SBUF ----
                xwh = gatherpool.tile([128, 2, DT, LC], BF16, tag="xwh",
                                      name="xwh")
                nc.sync.dma_start(
                    xwh[:],
                    rs_out[:].rearrange("s (p f t) -> p s f t", p=128, f=DT))

                # ---- halo K (window cols L=0:64, R=128:192) ----
                for g in range(DT):
                    for s, c0 in ((0, 0), (1, 128)):
                        for f in range(DT):
                            nc.tensor.matmul(
                                ps_k[g // 2][:, g % 2, c0:c0 + 64],
                                w_qkv[:, f * 1536 + 512 + g * 128:
                                      f * 1536 + 512 + (g + 1) * 128],
                                xwh[:, s, f, :], start=(f == 0),
                                stop=(f == DT - 1))
                    nc.vector.tensor_scalar_add(
                        KTg[g][:, 0:64], ps_k[g // 2][:, g % 2, 0:64],
                        pb[:, PB_QKVB + DT + g:PB_QKVB + DT + g + 1])
                    nc.vector.tensor_scalar_add(
                        KTg[g][:, 128:192], ps_k[g // 2][:, g % 2, 128:192],
                        pb[:, PB_QKVB + DT + g:PB_QKVB + DT + g + 1])

                # ---- halo V: L tokens -> Vt0[0:64], R tokens -> Vt1 ----
                for f in range(DT):
                    nc.tensor.matmul(
                        ps_v[0:64, :], xwh[:, 0, f, :],
                        w_qkv[:, f * 1536 + 1024:f * 1536 + 1536],
                        start=(f == 0), stop=(f == DT - 1))
                nc.vector.tensor_copy(Vt0[0:64, :], ps_v[0:64, :])
                ps_v2 = ppv.tile([128, D], F32, tag="ps_v")
                for f in range(DT):
                    nc.tensor.matmul(
                        ps_v2[0:64, :], xwh[:, 1, f, :],
                        w_qkv[:, f * 1536 + 1024:f * 1536 + 1536],
                        start=(f == 0), stop=(f == DT - 1))
                nc.vector.tensor_copy(Vt1[:], ps_v2[0:64, :])

                # ---- attention (per-head tiles so QK/softmax/AV pipeline) ----
                probs = []
                ps_sum = pp_ln.tile([1, H * LC], F32, tag="sums", name="ps_sum")
                for h in range(H):
                    hh, g = h % 2, h // 2
                    ps_l = ppatt.tile([128, 2, LC], F32, tag="ps_l", name="ps_l")
                    # kt0 = 128 keys (L+own); bias+mask accumulated by PE
                    nc.tensor.matmul(ps_l[:, 0, :],
                                     KTg[g][hh * DH:(hh + 1) * DH, 0:128],
                                     qT[g][hh * DH:(hh + 1) * DH, :],
                                     start=True, stop=False)
                    nc.tensor.matmul(ps_l[:, 0, :],
                                     bm[:, h * KW:h * KW + 128], ident_s[:],
                                     start=False, stop=True)
                    # kt1 = 64 keys (R), partitions 0:64
                    nc.tensor.matmul(ps_l[0:64, 1, :],
                                     KTg[g][hh * DH:(hh + 1) * DH, 128:192],
                                     qT[g][hh * DH:(hh + 1) * DH, :],
                                     start=True, stop=False)
                    nc.tensor.matmul(ps_l[0:64, 1, :],
                                     bm[:, h * KW + 128:h * KW + KW], ident_s[:],
                                     start=False, stop=True)
                    probs_h = actpool.tile([128, 2, LC], BF16, tag=f"probs{h}",
                                           name="probs_h")
                    nc.scalar.activation(probs_h[:, 0, :], ps_l[:, 0, :], AF.Exp)
                    nc.scalar.activation(probs_h[0:64, 1, :], ps_l[0:64, 1, :],
                                         AF.Exp)
                    probs.append(probs_h)
                    nc.tensor.matmul(ps_sum[0:1, h * LC:(h + 1) * LC],
                                     ones_b[:], probs_h[:, 0, :],
                                     start=True, stop=False)
                    nc.tensor.matmul(ps_sum[0:1, h * LC:(h + 1) * LC],
                                     ones_b[0:64, :], probs_h[0:64, 1, :],
                                     start=False, stop=True)
                # denominators -> bf16 reciprocal -> PE broadcast to DH rows
                rs_r = statpool.tile([1, H * LC], BF16, tag="rs_r")
                with nc.allow_low_precision(
                        reason="softmax 1/denom in bf16; error is ~0.4% of "
                               "the attention output, well inside tolerance"):
                    nc.vector.reciprocal(rs_r[:], ps_sum[:])
                rs_bc = ppbc.tile([DH, H * LC], F32, tag="bcast", name="rs_bc")
                nc.tensor.matmul(rs_bc[:], ones_row[0:1, 0:DH], rs_r[:],
                                 start=True, stop=True)
                # AV, one output tile per head-pair
                oT = [actpool.tile([128, LC], BF16, tag=f"oT{g}", name="oT_g")
                      for g in range(DT)]
                for h in range(H):
                    hh, g = h % 2, h // 2
                    ps_o = pp.tile([DH, LC], F32, tag="mm_out", name="ps_o")
                    nc.tensor.matmul(ps_o[:], Vt0[:, h * DH:(h + 1) * DH],
                                     probs[h][:, 0, :], start=True, stop=False)
                    nc.tensor.matmul(ps_o[:], Vt1[:, h * DH:(h + 1) * DH],
                                     probs[h][0:64, 1, :], start=False, stop=True)
                    nc.vector.tensor_mul(
                        oT[g][hh * DH:(hh + 1) * DH, :], ps_o[:],
                        rs_bc[:, h * LC:(h + 1) * LC])

                # ---- proj + residual ----
                for t in range(DT):
                    ps = pp.tile([128, LC], F32, tag="mm_out")
                    for f in range(DT):
                        nc.tensor.matmul(
                            ps[:],
                            w_proj[:, f * 512 + t * 128:f * 512 + (t + 1) * 128],
                            oT[f][:], start=(f == 0), stop=(f == DT - 1))
                    nc.vector.scalar_tensor_tensor(
                        hT[:, t, :], ps[:], pb[:, PB_PROJB + t:PB_PROJB + t + 1],
                        hT[:, t, :], op0=mybir.AluOpType.add,
                        op1=mybir.AluOpType.add)

                # ---- LN2 ----
                zb = actpool.tile([128, DT, LC], BF16, tag="zb")
                ln2_exp = layernorm(hT, 0, zb)

                # preload the gelu act table while ACT idles through the FF1
                # matmuls (pinned after LN2's Exp so the scheduler cannot
                # hoist it into the attention exps and thrash the table)
                gd = nc.scalar.activation(junk[:], ones_f[0:1, 0:1], AF.Gelu)
                add_dep_helper(gd.ins, ln2_exp.ins, sync=True,
                               reason="gelu table preload after last lnexp op")

                # ---- FF1 + gelu (z1 split in two tiles so FF2 can start
                # accumulating after the first half; per-feature bias injected
                # via K=1 matmul so each gelu covers 4 fo-tiles in one op) ----
                FH = FT // 2
                z1a = actpool.tile([128, FH, LC], BF16, tag="z1a")
                z1b = actpool.tile([128, FH, LC], BF16, tag="z1b")
                last_gelu = None
                for tq in range(FT // 4):
                    ps = pp.tile([128, 4, LC], F32, tag="mm_out", name="ps_ff1")
                    z1d = z1a if tq < 2 else z1b
                    for tt in range(4):
                        t = tq * 4 + tt
                        for f in range(DT):
                            nc.tensor.matmul(
                                ps[:, tt, :],
                                w_ff1[:, f * 2048 + t * 128:
                                      f * 2048 + (t + 1) * 128],
                                zb[:, f, :], start=(f == 0), stop=False)
                        nc.tensor.matmul(
                            ps[:, tt, :], fb[0:1, t * 128:(t + 1) * 128],
                            ones_row[0:1, 0:LC], start=False, stop=True)
                    last_gelu = nc.scalar.activation(
                        z1d[:, (tq % 2) * 4:(tq % 2) * 4 + 4, :], ps[:],
                        AF.Gelu)

                # switch the act table back while ACT idles through FF2
                # (pinned after the last gelu so it cannot interleave)
                if l + 1 < LAYERS:
                    ed = nc.scalar.activation(junk[:], ones_f[0:1, 0:1], AF.Exp)
                    add_dep_helper(ed.ins, last_gelu.ins, sync=True,
                                   reason="lnexp table preload after last gelu")

                # ---- FF2 + residual ----
                for t in range(DT):
                    ps = pp.tile([128, LC], F32, tag="mm_out")
                    for g in range(FT):
                        z1d = z1a if g < FH else z1b
                        nc.tensor.matmul(
                            ps[:],
                            w_ff2[:, g * 512 + t * 128:g * 512 + (t + 1) * 128],
                            z1d[:, g % FH, :], start=(g == 0), stop=(g == FT - 1))
                    nc.vector.scalar_tensor_tensor(
                        hT[:, t, :], ps[:], pb[:, PB_FF2B + t:PB_FF2B + t + 1],
                        hT[:, t, :], op0=mybir.AluOpType.add,
                        op1=mybir.AluOpType.add)

            # ---- output projection: y.T = tanh(out_w @ h.T + out_b) ----
            # (act table is gelu_and_others here, which contains Tanh)
            hb = actpool.tile([128, DT, LC], BF16, tag="hb")
            nc.vector.tensor_copy(hb[:], hT[:])
            ps_y = pp.tile([PATCH, LC], F32, tag="mm_out", name="ps_y")
            for f in range(DT):
                nc.tensor.matmul(ps_y[:], wout_s[:, f * PATCH:(f + 1) * PATCH],
                                 hb[:, f, :], start=(f == 0), stop=(f == DT - 1))
            y_sb = actpool.tile([PATCH, LC], F32, tag="y_sb")
            nc.scalar.activation(y_sb[:], ps_y[:], AF.Tanh,
                                 bias=outb_s[:, 0:1], scale=1.0)
            nc.sync.dma_start(yT[:], y_sb[:])

    nc.finalize()
    return nc


def _prep_inputs(inputs):
    """Host-side: pack full fp32 inputs into per-core in_maps."""
    I = {k: np.asarray(v, np.float32) for k, v in inputs.items()}

    scale = np.float32(DH ** -0.5)
    qkv_w = I["qkv_w"].copy()          # [LAYERS, 3D, D]
    qkv_b = I["qkv_b"].copy()          # [LAYERS, 3D]
    ff1_w = I["ff1_w"].copy()          # [LAYERS, DFF, D]
    ff1_b = I["ff1_b"].copy()          # [LAYERS, DFF]
    proj_b = I["proj_b"].copy()        # [LAYERS, D]
    # fold LN affines into the consuming matmuls (exact algebra, fp32):
    # (xn*g + b) @ W.T = xn @ (W*diag(g)).T + W@b
    for l in range(LAYERS):
        qkv_b[l] += qkv_w[l] @ I["ln1_b"][l]
        qkv_w[l] *= I["ln1_g"][l][None, :]
        ff1_b[l] += ff1_w[l] @ I["ln2_b"][l]
        ff1_w[l] *= I["ln2_g"][l][None, :]
        # v-bias passes through softmax-normalized attention exactly:
        # o = sum_k a_k (v_k + bv) = o' + bv  ->  fold W_o @ bv into proj_b
        proj_b[l] += I["proj_w"][l] @ qkv_b[l, 2 * D:]
    qkv_w[:, :D] *= scale
    qkv_b[:, :D] *= scale

    def part_major(m):
        # [X*128, Y] -> [128, X*Y] with column blocks per 128-row tile
        X = m.shape[0] // 128
        return np.ascontiguousarray(
            m.reshape(X, 128, m.shape[1]).transpose(1, 0, 2).reshape(128, -1))

    wblob = np.empty((LAYERS, 128, WCOLS), BF)
    pblob = np.empty((LAYERS, 128, PCOLS), np.float32)
    for l in range(LAYERS):
        qkvT = np.ascontiguousarray(qkv_w[l].T)          # [D, 3D]
        projT = np.ascontiguousarray(I["proj_w"][l].T)   # [D, D]
        ff1T = np.ascontiguousarray(ff1_w[l].T)          # [D, DFF]
        ff2T = np.ascontiguousarray(I["ff2_w"][l].T)     # [DFF, D]
        wblob[l, :, OFF_QKV:OFF_PROJ] = part_major(qkvT).astype(BF)
        wblob[l, :, OFF_PROJ:OFF_FF1] = part_major(projT).astype(BF)
        wblob[l, :, OFF_FF1:OFF_FF2] = part_major(ff1T).astype(BF)
        wblob[l, :, OFF_FF2:WCOLS] = part_major(ff2T).astype(BF)
        pblob[l, :, PB_QKVB:PB_QKVB + 12] = qkv_b[l].reshape(12, 128).T
        pblob[l, :, PB_PROJB:PB_PROJB + 4] = proj_b[l].reshape(4, 128).T
        pblob[l, :, PB_FF1B:PB_FF1B + 16] = ff1_b[l].reshape(16, 128).T
        pblob[l, :, PB_FF2B:PB_FF2B + 4] = I["ff2_b"][l].reshape(4, 128).T
    fbrow = ff1_b.astype(BF)                             # [LAYERS, DFF]

    # transposed attention bias+mask table over the per-core 192-key window
    # bmt_c[l, q, h, wk] = rpb[l, h, (g-q)+K-1] inside the clamped window
    # else NEG, where g = 64*(c-1) + wk is the global key index.
    i = np.arange(L)
    ni = np.clip(i - K // 2, 0, L - K)                   # [L] window starts
    rpb = I["rpb"]                                       # [LAYERS, H, 2K-1]

    w_in_T = np.ascontiguousarray(I["in_w"].T).astype(BF)          # [PATCH, D]
    inb = np.ascontiguousarray(I["in_b"].reshape(DT, 128).T)       # [128, DT]
    out_wT = np.ascontiguousarray(I["out_w"].T)                    # [D, PATCH]
    w_out = part_major(out_wT).astype(BF)                          # [128, 4*PATCH]
    outb = np.ascontiguousarray(I["out_b"].reshape(PATCH, 1))
    ident = np.eye(DH, dtype=BF)

    x_tok = I["x"].reshape(L, PATCH)                     # [L, PATCH]

    in_maps = []
    for c in range(NC):
        xT_c = np.ascontiguousarray(x_tok[c * LC:(c + 1) * LC].T).astype(BF)
        q = c * LC + np.arange(LC)                       # [LC] global queries
        g = 64 * (c - 1) + np.arange(KW)                 # [KW] global keys
        in_win = ((g[None, :] >= 0) & (g[None, :] < L)
                  & (g[None, :] >= ni[q][:, None])
                  & (g[None, :] < (ni[q] + K)[:, None]))             # [LC, KW]
        rel = np.clip(g[None, :] - q[:, None] + (K - 1), 0, 2 * K - 2)
        # [LAYERS, H, LC, KW] -> transpose to [LAYERS, LC(q), H, KW]
        bmt_c = np.where(in_win[None, None], rpb[:, :, rel],
                         np.float32(NEG)).transpose(0, 2, 1, 3)
        bmt_c = np.ascontiguousarray(
            bmt_c.reshape(LAYERS, LC, H * KW)).astype(BF)
        in_maps.append({
            "xT": xT_c,
            "w_in_T": w_in_T,
            "inb": inb,
            "wblob": wblob,
            "pblob": pblob,
            "fbrow": fbrow,
            "bmt": bmt_c,
            "ident": ident,
            "w_out": w_out,
            "outb": outb,
        })
    return in_maps


def kernel(**inputs):
    if "nc" not in _BUILD_CACHE:
        _BUILD_CACHE["nc"] = _build()
    nc = _BUILD_CACHE["nc"]
    in_maps = _prep_inputs(inputs)
    res = run_bass_kernel_spmd(nc, in_maps, core_ids=list(range(NC)))
    y = np.empty((1, 1, L * PATCH), np.float32)
    for c in range(NC):
        yT_c = res.results[c]["yT"]                      # [PATCH, LC]
        y[0, 0, c * LC * PATCH:(c + 1) * LC * PATCH] = yT_c.T.reshape(-1)
    return y


# revision 40
# speedup vs baseline: 1.2062x; 1.0078x over previous
"""Trainium2 Bass kernel for nn_AudioTransformer (neighborhood-attention transformer).

Strategy: sequence-parallel over 8 NeuronCores (64 tokens/core). Weights are
replicated per core in bf16 and streamed layer-by-layer (double-buffered,
issue split across engine DGE queues so no single queue serializes).
Activations live feature-major (features on SBUF partitions, tokens on the
free dim) so the whole layer stack runs without a single on-chip transpose.

Cross-core halo exchange per layer is a ReduceScatter: each core deposits its
LN1 output slab into the input slots of exactly the two neighbor cores that
need it (zeros elsewhere, pre-zeroed once at startup); the reduction collapses
the redundancy so the collective output is just the 2 halo slabs (128 tokens)
instead of a full 512-token AllGather — 18.3us vs 28.1us per layer.

The neighborhood attention runs dense over a 192-key window (ranks c-1,c,c+1)
with a host-precomputed transposed bias table (rel-pos bias inside the clamped
window, -60 outside) that is accumulated into the logits by the PE via an
identity matmul. Keys-on-partitions makes the softmax key-reduction a
ones-matmul; softmax skips max-subtraction (logits provably in [-2, 2]).
Q/K/V for the core's own tokens are computed during the collective.
"""

import numpy as np
import ml_dtypes

import concourse.bass as bass
import concourse.mybir as mybir
import concourse.tile as tile
from concourse.tile import add_dep_helper
from concourse import bacc
from concourse.bass_utils import run_bass_kernel_spmd


def _install_act_table_filter():
    """Make the act-table chooser resolve Ln/Exp/Identity/Copy only via the
    natural_log_exp_and_others set so each layer needs just 2 LUT swaps
    (to gelu_and_others and back) instead of 5. Positional set ids are
    preserved; sets are only shrunk, so every emitted load is still valid."""
    import concourse.bacc as _bacc_mod
    if getattr(_bacc_mod, "_ant_act_filter", False):
        return
    _orig = _bacc_mod.get_activation_tables
    A = mybir.ActivationFunctionType
    movable = {A.Ln, A.Exp, A.Identity, A.Copy}

    def _filtered(arch):
        t = _orig(arch)
        out = {}
        for name, funcs in t.items():
            if name == "natural_log_exp_and_others":
                out[name] = set(funcs)
            else:
                out[name] = set(funcs) - movable
        return out

    _bacc_mod.get_activation_tables = _filtered
    _bacc_mod._ant_act_filter = True

BF = ml_dtypes.bfloat16
F32 = mybir.dt.float32
BF16 = mybir.dt.bfloat16

NC = 8          # cores
L = 512         # total tokens
LC = L // NC    # tokens per core = 64
D = 512         # model dim
DT = D // 128   # 4 feature tiles
H = 8           # heads
DH = 64         # head dim
DFF = 2048      # ff dim
FT = DFF // 128  # 16 ff tiles
PATCH = 32
LAYERS = 8
K = 127         # neighborhood size
KW = 192        # per-core key window: ranks c-1, c, c+1 (64 each)
NEG = -60.0     # out-of-window logit bias (exp(-60+2) == 0 in fp32/bf16)
SLAB = D * LC   # one x~ slab, elements

# wblob column offsets (per 128-row partition, bf16)
OFF_QKV = 0            # 4 fi-tiles x 1536
OFF_PROJ = 6144        # 4 fi-tiles x 512
OFF_FF1 = 8192         # 4 fi-tiles x 2048
OFF_FF2 = 16384        # 16 fi-tiles x 512
WCOLS = 24576

# pblob columns (f32)
PB_QKVB = 0    # 12 (q 4, k 4, v 4 -- v unused on device, folded into proj_b)
PB_PROJB = 12  # 4
PB_FF1B = 16   # 16
PB_FF2B = 32   # 4
PCOLS = 36

_BUILD_CACHE = {}


def _build():
    """Build + finalize the SPMD Bass graph (same graph on all 8 cores)."""
    _install_act_table_filter()
    nc = bacc.Bacc(None, target_bir_lowering=False)
    AF = mybir.ActivationFunctionType

    # ---- DRAM parameters (per-core inputs) ----
    xT = nc.dram_tensor("xT", [PATCH, LC], BF16, kind="ExternalInput")
    w_in_T = nc.dram_tensor("w_in_T", [PATCH, D], BF16, kind="ExternalInput")
    inb = nc.dram_tensor("inb", [128, DT], F32, kind="ExternalInput")
    wblob = nc.dram_tensor("wblob", [LAYERS, 128, WCOLS], BF16, kind="ExternalInput")
    pblob = nc.dram_tensor("pblob", [LAYERS, 128, PCOLS], F32, kind="ExternalInput")
    fbrow = nc.dram_tensor("fbrow", [LAYERS, DFF], BF16, kind="ExternalInput")
    bmt = nc.dram_tensor("bmt", [LAYERS, DH, H * KW], BF16, kind="ExternalInput")
    ident = nc.dram_tensor("ident", [DH, DH], BF16, kind="ExternalInput")
    w_out = nc.dram_tensor("w_out", [128, 128], BF16, kind="ExternalInput")
    outb = nc.dram_tensor("outb", [PATCH, 1], F32, kind="ExternalInput")
    yT = nc.dram_tensor("yT", [PATCH, LC], F32, kind="ExternalOutput")

    with tile.TileContext(nc) as tc:
        with (
            tc.tile_pool(name="singles", bufs=1) as singles,
            tc.tile_pool(name="wpool", bufs=2) as wpool,
            tc.tile_pool(name="ppool", bufs=2) as ppool,
            tc.tile_pool(name="bmpool", bufs=2) as bmpool,
            tc.tile_pool(name="actpool", bufs=2) as actpool,
            tc.tile_pool(name="gatherpool", bufs=2) as gatherpool,
            tc.tile_pool(name="tmppool", bufs=3) as tmppool,
            tc.tile_pool(name="statpool", bufs=4) as statpool,
            tc.tile_pool(name="agdram", bufs=2, space="DRAM") as agdram,
            tc.tile_pool(name="agdram1", bufs=1, space="DRAM") as agdram1,
            # PSUM: 8 banks total, every tile slot rounds to one bank.
            # pp:mm_out(3) + pp_ln:sums(1) + ppv(1) + ppatt:ps_l(2) + ppbc(1) = 8
            tc.tile_pool(name="pp", bufs=3, space="PSUM") as pp,
            tc.tile_pool(name="pp_ln", bufs=1, space="PSUM") as pp_ln,
            tc.tile_pool(name="ppv", bufs=1, space="PSUM") as ppv,
            tc.tile_pool(name="ppatt", bufs=2, space="PSUM") as ppatt,
            tc.tile_pool(name="ppbc", bufs=1, space="PSUM") as ppbc,
        ):
            # persistent tiles
            hT = singles.tile([128, DT, LC], F32)          # residual stream h.T
            ones_f = singles.tile([128, 1], F32)
            ones_b = singles.tile([128, 1], BF16)
            ones_row = singles.tile([1, 128], BF16)
            ones_bcf = singles.tile([1, 128], F32)
            xin = singles.tile([PATCH, LC], BF16)
            win = singles.tile([PATCH, D], BF16)
            inb_s = singles.tile([128, DT], F32)
            wout_s = singles.tile([128, 128], BF16)
            outb_s = singles.tile([PATCH, 1], F32)
            ident_s = singles.tile([DH, DH], BF16)
            zsb = singles.tile([128, 16, 256], BF16)       # zero fill source
            junk = singles.tile([1, 1], F32)               # act-table preload out

            nc.vector.memset(ones_f[:], 1.0)
            nc.vector.memset(ones_b[:], 1.0)
            nc.vector.memset(ones_row[:], 1.0)
            nc.vector.memset(ones_bcf[:], 1.0)
            nc.vector.memset(zsb[:], 0.0)
            nc.sync.dma_start(xin[:], xT[:])
            nc.sync.dma_start(win[:], w_in_T[:])
            nc.sync.dma_start(inb_s[:], inb[:])
            nc.scalar.dma_start(wout_s[:], w_out[:])
            nc.scalar.dma_start(outb_s[:], outb[:])
            nc.scalar.dma_start(ident_s[:], ident[:])

            # two persistent RS input buffers (one per layer parity), zeroed
            # once at startup; only the two per-layer halo slabs are ever
            # rewritten, so the zero slots stay zero for the whole run.
            rs_bufs = []
            for i, q in enumerate((nc.gpsimd, nc.scalar)):
                rs_b = agdram1.tile([16, 128, 256], BF16, tag=f"rs_in{i}",
                                    name="rs_b")
                q.dma_start(rs_b[:].rearrange("c p t -> p c t"), zsb[:])
                rs_bufs.append(rs_b)

            def layernorm(src, gcol, dst):
                """src [128,DT,LC] f32 -> dst [128,DT,LC] bf16 (normalized;
                gamma/beta folded into consumer matmul weights on host).
                Returns the final ACT (Exp) instruction for dep pinning."""
                ps_s = pp_ln.tile([1, 2 * LC], F32, tag="sums", name="ps_s")
                for f in range(DT):
                    nc.tensor.matmul(ps_s[0:1, 0:LC], ones_f[:], src[:, f, :],
                                     start=(f == 0), stop=(f == DT - 1))
                sq = tmppool.tile([128, DT, LC], F32, tag="ln_sq")
                nc.vector.tensor_mul(sq[:], src[:], src[:])
                for f in range(DT):
                    nc.tensor.matmul(ps_s[0:1, LC:2 * LC], ones_f[:], sq[:, f, :],
                                     start=(f == 0), stop=(f == DT - 1))
                st = statpool.tile([1, 2 * LC], F32, tag="ln_st")
                nc.vector.tensor_scalar_mul(st[0:1, 0:LC], ps_s[0:1, 0:LC], 1.0 / D)
                # (m2 - eps) elementwise, then var+eps = sumsq/D - (m2 - eps)
                m2 = statpool.tile([1, LC], F32, tag="ln_m2")
                nc.vector.tensor_mul(m2[:], st[0:1, 0:LC], st[0:1, 0:LC])
                nc.vector.tensor_scalar_add(m2[:], m2[:], -1e-5)
                var = statpool.tile([1, LC], F32, tag="ln_var")
                nc.vector.scalar_tensor_tensor(
                    var[:], ps_s[0:1, LC:2 * LC], 1.0 / D, m2[:],
                    op0=mybir.AluOpType.mult, op1=mybir.AluOpType.subtract)
                # rstd = exp(-0.5*ln(var)) -- keeps ACT in the Ln/Exp func set
                sd = statpool.tile([1, LC], F32, tag="ln_sd")
                nc.scalar.activation(sd[:], var[:], AF.Ln)
                exp_i = nc.scalar.activation(st[0:1, LC:2 * LC], sd[:], AF.Exp,
                                             scale=-0.5)
                # broadcast (mean, rstd) across all 128 partitions via K=1 matmul
                bc = ppbc.tile([128, 2 * LC], F32, tag="bcast", name="bc")
                nc.tensor.matmul(bc[:], ones_bcf[:], st[:], start=True, stop=True)
                t0 = tmppool.tile([128, DT, LC], F32, tag="ln_t0")
                mean_w = bc[:, 0:LC].unsqueeze(1).to_broadcast([128, DT, LC])
                rstd_w = bc[:, LC:2 * LC].unsqueeze(1).to_broadcast([128, DT, LC])
                nc.vector.tensor_sub(t0[:], src[:], mean_w)
                nc.vector.tensor_mul(dst[:], t0[:], rstd_w)
                return exp_i

            # ---- input projection: h0.T = in_w @ x_slice.T + in_b ----
            for t in range(DT):
                ps = pp.tile([128, LC], F32, tag="mm_out")
                nc.tensor.matmul(ps[:], win[:, t * 128:(t + 1) * 128], xin[:],
                                 start=True, stop=True)
                nc.scalar.activation(hT[:, t, :], ps[:], AF.Identity,
                                     bias=inb_s[:, t:t + 1], scale=1.0)

            # RS input slot chunks (chunk k = slot k//2, pos k%2):
            #   right-send -> (rank+1, pos0), rank 7 redirects to (0, pos0)
            #   left-send  -> (rank-1, pos1), rank 0 redirects to (7, pos1)
            # both redirect targets are halo positions nobody reads.
            rank = nc.sync.partition_id()
            ch_right = 2 * ((rank + 1) * (rank <= 6))
            rank_g = nc.gpsimd.partition_id()
            ch_left = 2 * ((rank_g - 1) * (rank_g >= 1) + 7 * (rank_g <= 0)) + 1

            def load_layer(l, queues):
                """Stream layer l's params; issue split across DGE queues."""
                w_qkv = wpool.tile([128, 6144], BF16, tag="w_qkv", name="w_qkv")
                w_proj = wpool.tile([128, 2048], BF16, tag="w_proj", name="w_proj")
                w_ff1 = wpool.tile([128, 8192], BF16, tag="w_ff1", name="w_ff1",
                                   bufs=3)
                w_ff2 = wpool.tile([128, 8192], BF16, tag="w_ff2", name="w_ff2",
                                   bufs=3)
                pb = ppool.tile([128, PCOLS], F32, tag="pb", name="pb")
                fb = ppool.tile([1, DFF], BF16, tag="fb", name="fb")
                bm = bmpool.tile([DH, H * KW], BF16, tag="bm", name="bm")
                qs, qa = queues
                qs.dma_start(w_qkv[:], wblob[l, :, OFF_QKV:OFF_PROJ])
                qs.dma_start(w_proj[:], wblob[l, :, OFF_PROJ:OFF_FF1])
                qs.dma_start(w_ff1[:], wblob[l, :, OFF_FF1:OFF_FF2])
                qs.dma_start(pb[:], pblob[l])
                qs.dma_start(w_ff2[:], wblob[l, :, OFF_FF2:WCOLS])
                qa.dma_start(fb[:], fbrow[l].unsqueeze(0))
                qa.dma_start(bm[:], bmt[l])
                return w_qkv, w_proj, w_ff1, w_ff2, pb, fb, bm

            cur = load_layer(0, (nc.sync, nc.scalar))
            for l in range(LAYERS):
                w_qkv, w_proj, w_ff1, w_ff2, pb, fb, bm = cur

                # ---- LN1 ----
                xb = actpool.tile([128, DT, LC], BF16, tag="xb")
                layernorm(hT, PB_QKVB, xb)

                # ---- halo slab deposit + ReduceScatter ----
                rs_in = rs_bufs[l % 2]
                nc.sync.dma_start(
                    rs_in[bass.ds(ch_right, 1)]
                    .rearrange("c p (f t) -> p (c f) t", f=DT), xb[:])
                nc.gpsimd.dma_start(
                    rs_in[bass.ds(ch_left, 1)]
                    .rearrange("c p (f t) -> p (c f) t", f=DT), xb[:])
                rs_out = agdram.tile([2, SLAB], BF16, tag="rs_out",
                                     name="rs_out")
                with nc.allow_low_precision(
                        reason="exact: every summed region has a single "
                               "nonzero contributor, the rest are zeros"):
                    nc.gpsimd.collective_compute(
                        "ReduceScatter", mybir.AluOpType.add,
                        ins=[rs_in[:].opt()], outs=[rs_out[:].opt()],
                        replica_groups=[list(range(NC))])

                # prefetch next layer's weights NOW: the transfers stream
                # during the collective on idle queues.
                if l + 1 < LAYERS:
                    cur = load_layer(l + 1, (nc.sync, nc.scalar))

                # ---- during-RS compute: Q, own-K, own-V (local xb only) ----
                qT = []
                for t in range(DT):
                    ps = pp.tile([128, LC], F32, tag="mm_out")
                    for f in range(DT):
                        nc.tensor.matmul(
                            ps[:],
                            w_qkv[:, f * 1536 + t * 128:f * 1536 + (t + 1) * 128],
                            xb[:, f, :], start=(f == 0), stop=(f == DT - 1))
                    qT_t = actpool.tile([128, LC], BF16, tag=f"qT{t}", name="qT_t")
                    nc.vector.tensor_scalar_add(
                        qT_t[:], ps[:], pb[:, PB_QKVB + t:PB_QKVB + t + 1])
                    qT.append(qT_t)

                ps_k = [ppatt.tile([128, 2, KW], F32, tag="ps_l", name="ps_k")
                        for _ in range(2)]
                KTg = [gatherpool.tile([128, KW], BF16, tag=f"KTg{g}",
                                       name="KTg_g") for g in range(DT)]
                for g in range(DT):
                    for f in range(DT):
                        nc.tensor.matmul(
                            ps_k[g // 2][:, g % 2, 64:128],
                            w_qkv[:, f * 1536 + 512 + g * 128:
                                  f * 1536 + 512 + (g + 1) * 128],
                            xb[:, f, :], start=(f == 0), stop=(f == DT - 1))
                    nc.vector.tensor_scalar_add(
                        KTg[g][:, 64:128], ps_k[g // 2][:, g % 2, 64:128],
                        pb[:, PB_QKVB + DT + g:PB_QKVB + DT + g + 1])

                Vt0 = gatherpool.tile([128, D], BF16, tag="Vt0", name="Vt0")
                Vt1 = gatherpool.tile([DH, D], BF16, tag="Vt1", name="Vt1")
                ps_v = ppv.tile([128, D], F32, tag="ps_v")
                for f in range(DT):
                    nc.tensor.matmul(
                        ps_v[64:128, :], xb[:, f, :],
                        w_qkv[:, f * 1536 + 1024:f * 1536 + 1536],
                        start=(f == 0), stop=(f == DT - 1))
                nc.vector.tensor_copy(Vt0[64:128, :], ps_v[64:128, :])

                # ---- gathered halo slabs -> SBUF ----
                xwh = gatherpool.tile([128, 2, DT, LC], BF16, tag="xwh",
                                      name="xwh")
                nc.scalar.dma_start(
                    xwh[:],
                    rs_out[:].rearrange("s (p f t) -> p s f t", p=128, f=DT))

                # ---- halo K (window cols L=0:64, R=128:192) ----
                for g in range(DT):
                    for s, c0 in ((0, 0), (1, 128)):
                        for f in range(DT):
                            nc.tensor.matmul(
                                ps_k[g // 2][:, g % 2, c0:c0 + 64],
                                w_qkv[:, f * 1536 + 512 + g * 128:
                                      f * 1536 + 512 + (g + 1) * 128],
                                xwh[:, s, f, :], start=(f == 0),
                                stop=(f == DT - 1))
                    nc.vector.tensor_scalar_add(
                        KTg[g][:, 0:64], ps_k[g // 2][:, g % 2, 0:64],
                        pb[:, PB_QKVB + DT + g:PB_QKVB + DT + g + 1])
                    nc.vector.tensor_scalar_add(
                        KTg[g][:, 128:192], ps_k[g // 2][:, g % 2, 128:192],
                        pb[:, PB_QKVB + DT + g:PB_QKVB + DT + g + 1])

                # ---- halo V: L tokens -> Vt0[0:64], R tokens -> Vt1 ----
                for f in range(DT):
                    nc.tensor.matmul(
                        ps_v[0:64, :], xwh[:, 0, f, :],
                        w_qkv[:, f * 1536 + 1024:f * 1536 + 1536],
                        start=(f == 0), stop=(f == DT - 1))
                nc.vector.tensor_copy(Vt0[0:64, :], ps_v[0:64, :])
                ps_v2 = ppv.tile([128, D], F32, tag="ps_v")
                for f in range(DT):
                    nc.tensor.matmul(
                        ps_v2[0:64, :], xwh[:, 1, f, :],
                        w_qkv[:, f * 1536 + 1024:f * 1536 + 1536],
                        start=(f == 0), stop=(f == DT - 1))
                nc.vector.tensor_copy(Vt1[:], ps_v2[0:64, :])

                # ---- attention (per-head tiles so QK/softmax/AV pipeline) ----
                probs = []
                ps_sum = pp_ln.tile([1, H * LC], F32, tag="sums", name="ps_sum")
                for h in range(H):
                    hh, g = h % 2, h // 2
                    ps_l = ppatt.tile([128, 2, LC], F32, tag="ps_l", name="ps_l")
                    # kt0 = 128 keys (L+own); bias+mask accumulated by PE
                    nc.tensor.matmul(ps_l[:, 0, :],
                                     KTg[g][hh * DH:(hh + 1) * DH, 0:128],
                                     qT[g][hh * DH:(hh + 1) * DH, :],
                                     start=True, stop=False)
                    nc.tensor.matmul(ps_l[:, 0, :],
                                     bm[:, h * KW:h * KW + 128], ident_s[:],
                                     start=False, stop=True)
                    # kt1 = 64 keys (R), partitions 0:64
                    nc.tensor.matmul(ps_l[0:64, 1, :],
                                     KTg[g][hh * DH:(hh + 1) * DH, 128:192],
                                     qT[g][hh * DH:(hh + 1) * DH, :],
                                     start=True, stop=False)
                    nc.tensor.matmul(ps_l[0:64, 1, :],
                                     bm[:, h * KW + 128:h * KW + KW], ident_s[:],
                                     start=False, stop=True)
                    probs_h = actpool.tile([128, 2, LC], BF16, tag=f"probs{h}",
                                           name="probs_h")
                    nc.scalar.activation(probs_h[:, 0, :], ps_l[:, 0, :], AF.Exp)
                    nc.scalar.activation(probs_h[0:64, 1, :], ps_l[0:64, 1, :],
                                         AF.Exp)
                    probs.append(probs_h)
                    nc.tensor.matmul(ps_sum[0:1, h * LC:(h + 1) * LC],
                                     ones_b[:], probs_h[:, 0, :],
                                     start=True, stop=False)
                    nc.tensor.matmul(ps_sum[0:1, h * LC:(h + 1) * LC],
                                     ones_b[0:64, :], probs_h[0:64, 1, :],
                                     start=False, stop=True)
                # denominators -> bf16 reciprocal -> PE broadcast to DH rows
                rs_r = statpool.tile([1, H * LC], BF16, tag="rs_r")
                with nc.allow_low_precision(
                        reason="softmax 1/denom in bf16; error is ~0.4% of "
                               "the attention output, well inside tolerance"):
                    nc.vector.reciprocal(rs_r[:], ps_sum[:])
                rs_bc = ppbc.tile([DH, H * LC], F32, tag="bcast", name="rs_bc")
                nc.tensor.matmul(rs_bc[:], ones_row[0:1, 0:DH], rs_r[:],
                                 start=True, stop=True)
                # AV, one output tile per head-pair
                oT = [actpool.tile([128, LC], BF16, tag=f"oT{g}", name="oT_g")
                      for g in range(DT)]
                for h in range(H):
                    hh, g = h % 2, h // 2
                    ps_o = pp.tile([DH, LC], F32, tag="mm_out", name="ps_o")
                    nc.tensor.matmul(ps_o[:], Vt0[:, h * DH:(h + 1) * DH],
                                     probs[h][:, 0, :], start=True, stop=False)
                    nc.tensor.matmul(ps_o[:], Vt1[:, h * DH:(h + 1) * DH],
                                     probs[h][0:64, 1, :], start=False, stop=True)
                    nc.vector.tensor_mul(
                        oT[g][hh * DH:(hh + 1) * DH, :], ps_o[:],
                        rs_bc[:, h * LC:(h + 1) * LC])

                # ---- proj + residual ----
                for t in range(DT):
                    ps = pp.tile([128, LC], F32, tag="mm_out")
                    for f in range(DT):
                        nc.tensor.matmul(
                            ps[:],
                            w_proj[:, f * 512 + t * 128:f * 512 + (t + 1) * 128],
                            oT[f][:], start=(f == 0), stop=(f == DT - 1))
                    nc.vector.scalar_tensor_tensor(
                        hT[:, t, :], ps[:], pb[:, PB_PROJB + t:PB_PROJB + t + 1],
                        hT[:, t, :], op0=mybir.AluOpType.add,
                        op1=mybir.AluOpType.add)

                # ---- LN2 ----
                zb = actpool.tile([128, DT, LC], BF16, tag="zb")
                ln2_exp = layernorm(hT, 0, zb)

                # preload the gelu act table while ACT idles through the FF1
                # matmuls (pinned after LN2's Exp so the scheduler cannot
                # hoist it into the attention exps and thrash the table)
                gd = nc.scalar.activation(junk[:], ones_f[0:1, 0:1], AF.Gelu)
                add_dep_helper(gd.ins, ln2_exp.ins, sync=True,
                               reason="gelu table preload after last lnexp op")

                # ---- FF1 + gelu (z1 split in two tiles so FF2 can start
                # accumulating after the first half; per-feature bias injected
                # via K=1 matmul so each gelu covers 4 fo-tiles in one op) ----
                FH = FT // 2
                z1a = actpool.tile([128, FH, LC], BF16, tag="z1a")
                z1b = actpool.tile([128, FH, LC], BF16, tag="z1b")
                last_gelu = None
                for tq in range(FT // 4):
                    ps = pp.tile([128, 4, LC], F32, tag="mm_out", name="ps_ff1")
                    z1d = z1a if tq < 2 else z1b
                    for tt in range(4):
                        t = tq * 4 + tt
                        for f in range(DT):
                            nc.tensor.matmul(
                                ps[:, tt, :],
                                w_ff1[:, f * 2048 + t * 128:
                                      f * 2048 + (t + 1) * 128],
                                zb[:, f, :], start=(f == 0), stop=False)
                        nc.tensor.matmul(
                            ps[:, tt, :], fb[0:1, t * 128:(t + 1) * 128],
                            ones_row[0:1, 0:LC], start=False, stop=True)
                    last_gelu = nc.scalar.activation(
                        z1d[:, (tq % 2) * 4:(tq % 2) * 4 + 4, :], ps[:],
                        AF.Gelu)

                # switch the act table back while ACT idles through FF2
                # (pinned after the last gelu so it cannot interleave)
                if l + 1 < LAYERS:
                    ed = nc.scalar.activation(junk[:], ones_f[0:1, 0:1], AF.Exp)
                    add_dep_helper(ed.ins, last_gelu.ins, sync=True,
                                   reason="lnexp table preload after last gelu")

                # ---- FF2 + residual ----
                for t in range(DT):
                    ps = pp.tile([128, LC], F32, tag="mm_out")
                    for g in range(FT):
                        z1d = z1a if g < FH else z1b
                        nc.tensor.matmul(
                            ps[:],
                            w_ff2[:, g * 512 + t * 128:g * 512 + (t + 1) * 128],
                            z1d[:, g % FH, :], start=(g == 0), stop=(g == FT - 1))
                    nc.vector.scalar_tensor_tensor(
                        hT[:, t, :], ps[:], pb[:, PB_FF2B + t:PB_FF2B + t + 1],
                        hT[:, t, :], op0=mybir.AluOpType.add,
                        op1=mybir.AluOpType.add)

            # ---- output projection: y.T = tanh(out_w @ h.T + out_b) ----
            # (act table is gelu_and_others here, which contains Tanh)
            hb = actpool.tile([128, DT, LC], BF16, tag="hb")
            nc.vector.tensor_copy(hb[:], hT[:])
            ps_y = pp.tile([PATCH, LC], F32, tag="mm_out", name="ps_y")
            for f in range(DT):
                nc.tensor.matmul(ps_y[:], wout_s[:, f * PATCH:(f + 1) * PATCH],
                                 hb[:, f, :], start=(f == 0), stop=(f == DT - 1))
            y_sb = actpool.tile([PATCH, LC], F32, tag="y_sb")
            nc.scalar.activation(y_sb[:], ps_y[:], AF.Tanh,
                                 bias=outb_s[:, 0:1], scale=1.0)
            nc.sync.dma_start(yT[:], y_sb[:])

    nc.finalize()
    return nc


def _prep_inputs(inputs):
    """Host-side: pack full fp32 inputs into per-core in_maps."""
    I = {k: np.asarray(v, np.float32) for k, v in inputs.items()}

    scale = np.float32(DH ** -0.5)
    qkv_w = I["qkv_w"].copy()          # [LAYERS, 3D, D]
    qkv_b = I["qkv_b"].copy()          # [LAYERS, 3D]
    ff1_w = I["ff1_w"].copy()          # [LAYERS, DFF, D]
    ff1_b = I["ff1_b"].copy()          # [LAYERS, DFF]
    proj_b = I["proj_b"].copy()        # [LAYERS, D]
    # fold LN affines into the consuming matmuls (exact algebra, fp32):
    # (xn*g + b) @ W.T = xn @ (W*diag(g)).T + W@b
    for l in range(LAYERS):
        qkv_b[l] += qkv_w[l] @ I["ln1_b"][l]
        qkv_w[l] *= I["ln1_g"][l][None, :]
        ff1_b[l] += ff1_w[l] @ I["ln2_b"][l]
        ff1_w[l] *= I["ln2_g"][l][None, :]
        # v-bias passes through softmax-normalized attention exactly:
        # o = sum_k a_k (v_k + bv) = o' + bv  ->  fold W_o @ bv into proj_b
        proj_b[l] += I["proj_w"][l] @ qkv_b[l, 2 * D:]
    qkv_w[:, :D] *= scale
    qkv_b[:, :D] *= scale

    def part_major(m):
        # [X*128, Y] -> [128, X*Y] with column blocks per 128-row tile
        X = m.shape[0] // 128
        return np.ascontiguousarray(
            m.reshape(X, 128, m.shape[1]).transpose(1, 0, 2).reshape(128, -1))

    wblob = np.empty((LAYERS, 128, WCOLS), BF)
    pblob = np.empty((LAYERS, 128, PCOLS), np.float32)
    for l in range(LAYERS):
        qkvT = np.ascontiguousarray(qkv_w[l].T)          # [D, 3D]
        projT = np.ascontiguousarray(I["proj_w"][l].T)   # [D, D]
        ff1T = np.ascontiguousarray(ff1_w[l].T)          # [D, DFF]
        ff2T = np.ascontiguousarray(I["ff2_w"][l].T)     # [DFF, D]
        wblob[l, :, OFF_QKV:OFF_PROJ] = part_major(qkvT).astype(BF)
        wblob[l, :, OFF_PROJ:OFF_FF1] = part_major(projT).astype(BF)
        wblob[l, :, OFF_FF1:OFF_FF2] = part_major(ff1T).astype(BF)
        wblob[l, :, OFF_FF2:WCOLS] = part_major(ff2T).astype(BF)
        pblob[l, :, PB_QKVB:PB_QKVB + 12] = qkv_b[l].reshape(12, 128).T
        pblob[l, :, PB_PROJB:PB_PROJB + 4] = proj_b[l].reshape(4, 128).T
        pblob[l, :, PB_FF1B:PB_FF1B + 16] = ff1_b[l].reshape(16, 128).T
        pblob[l, :, PB_FF2B:PB_FF2B + 4] = I["ff2_b"][l].reshape(4, 128).T
    fbrow = ff1_b.astype(BF)                             # [LAYERS, DFF]

    # transposed attention bias+mask table over the per-core 192-key window
    # bmt_c[l, q, h, wk] = rpb[l, h, (g-q)+K-1] inside the clamped window
    # else NEG, where g = 64*(c-1) + wk is the global key index.
    i = np.arange(L)
    ni = np.clip(i - K // 2, 0, L - K)                   # [L] window starts
    rpb = I["rpb"]                                       # [LAYERS, H, 2K-1]

    w_in_T = np.ascontiguousarray(I["in_w"].T).astype(BF)          # [PATCH, D]
    inb = np.ascontiguousarray(I["in_b"].reshape(DT, 128).T)       # [128, DT]
    out_wT = np.ascontiguousarray(I["out_w"].T)                    # [D, PATCH]
    w_out = part_major(out_wT).astype(BF)                          # [128, 4*PATCH]
    outb = np.ascontiguousarray(I["out_b"].reshape(PATCH, 1))
    ident = np.eye(DH, dtype=BF)

    x_tok = I["x"].reshape(L, PATCH)                     # [L, PATCH]

    in_maps = []
    for c in range(NC):
        xT_c = np.ascontiguousarray(x_tok[c * LC:(c + 1) * LC].T).astype(BF)
        q = c * LC + np.arange(LC)                       # [LC] global queries
        g = 64 * (c - 1) + np.arange(KW)                 # [KW] global keys
        in_win = ((g[None, :] >= 0) & (g[None, :] < L)
                  & (g[None, :] >= ni[q][:, None])
                  & (g[None, :] < (ni[q] + K)[:, None]))             # [LC, KW]
        rel = np.clip(g[None, :] - q[:, None] + (K - 1), 0, 2 * K - 2)
        # [LAYERS, H, LC, KW] -> transpose to [LAYERS, LC(q), H, KW]
        bmt_c = np.where(in_win[None, None], rpb[:, :, rel],
                         np.float32(NEG)).transpose(0, 2, 1, 3)
        bmt_c = np.ascontiguousarray(
            bmt_c.reshape(LAYERS, LC, H * KW)).astype(BF)
        in_maps.append({
            "xT": xT_c,
            "w_in_T": w_in_T,
            "inb": inb,
            "wblob": wblob,
            "pblob": pblob,
            "fbrow": fbrow,
            "bmt": bmt_c,
            "ident": ident,
            "w_out": w_out,
            "outb": outb,
        })
    return in_maps


def kernel(**inputs):
    if "nc" not in _BUILD_CACHE:
        _BUILD_CACHE["nc"] = _build()
    nc = _BUILD_CACHE["nc"]
    in_maps = _prep_inputs(inputs)
    res = run_bass_kernel_spmd(nc, in_maps, core_ids=list(range(NC)))
    y = np.empty((1, 1, L * PATCH), np.float32)
    for c in range(NC):
        yT_c = res.results[c]["yT"]                      # [PATCH, LC]
        y[0, 0, c * LC * PATCH:(c + 1) * LC * PATCH] = yT_c.T.reshape(-1)
    return y
